# revision 48
# baseline (speedup 1.0000x reference)
"""Trainium2 Bass kernel for nn_MultiHeadAttention_KT (causal linear attention).

Math (per batch b):
  q' = leaky((q*qm) @ Wq + bq); k' = leaky((k*km) @ Wk + bk); v' = (v*vm) @ Wv
  per head h (DEPTH=64):   S_t = sum_{s<=t} k_s v_s^T ; z_t = sum_{s<=t} k_s
                           attn_t = (q_t @ S_t) / (q_t . z_t)
  out = concat_heads(attn) @ Wo + bo
Sharding: 8 cores = 2 batches x 4 head-groups (4 heads / 256 cols each).
Host transposes + bf16-casts inputs (xq = (q*qm)^T etc.); host sums the 4
partial output projections per batch (po is this core's heads' Wo slice).

All matmul operands are bf16 (1 cyc/row on PE + fast weight load); PSUM
accumulation stays f32.  Chunked linear attention (chunk C=128):
  AT   = K Q^T (per chunk, [s,t] layout)      masked with triu (s<=t)
  num  = ATm^T V_aug + Q S_aug                (V_aug = [V | 1], S_aug = [S | z])
  attn = num[:, :64] * (1/num[:, 64])
  S_aug += K_chunk^T V_aug                    (delta matmul; f32 master state
                                               on DVE + bf16 mirror)

q' is stored in per-head zero-padded tiles (qTz[jb][hh], [128, S] with the
other head's 64 partitions zeroed) so every attention matmul has a 128-tall
stationary: scores share one kT LDWEIGHTS for both heads and all stationaries
qualify for fast weight load, which keeps the small-matmul issue rate near
the stream rate.  Projection / output-projection units are pumped one at a
time between attention stages so the PE HAM clock gate stays open; the last
group's V projection and the previous group's output projection are held
back to feed the tail, where no other big matmuls remain.
"""

import os
import sys

sys.path.insert(0, "/opt/trn_rl_repo")

import ml_dtypes
import numpy as np

BF16 = np.dtype(ml_dtypes.bfloat16)

B, S, D, H = 2, 2048, 1024, 16
DEPTH = 64
N_CORES = 8
HPC = 4                 # heads per core
JS = HPC * DEPTH        # 256 projected columns per core
C = 128                 # attention chunk length
NCH = S // C            # 16 chunks
IB = D // 128           # 8 contraction blocks
SCH = 512               # projection s-chunk
NSC = S // SCH          # 4 projection chunks
JAUG = DEPTH + 1        # 65 (V augmented with ones column)
OSC = 512               # output projection s-chunk
CPO = OSC // C          # attention chunks per output chunk

MM_DTYPE = "bf16"       # informational (printed by test harness)
SIM_NO_PRELU = os.environ.get("KT_SIM_NO_PRELU") == "1"  # CoreSim lacks Prelu
TRACE = False           # set True from test harness to capture NTFF profile
TRACE_CORES = None
LAST_RESULTS = None     # BassKernelResults of the last kernel() call

_PROG = None


def _build():
    import concourse.bacc as bacc
    import concourse.mybir as mybir
    import concourse.tile as tile

    dt = mybir.dt
    f32 = dt.float32
    bf = dt.bfloat16
    AF = mybir.ActivationFunctionType
    Alu = mybir.AluOpType

    nc = bacc.Bacc("TRN2", target_bir_lowering=False, debug=False,
                   num_devices=N_CORES)

    # host pre-tiles x and weights so every DMA is per-partition contiguous.
    # xq/xk are half-major and xv quarter-major so the startup split loads
    # are single 4KB/2KB descriptor rows instead of 512B fragments (the DMA
    # engines are descriptor-rate bound at kernel start).
    HA = SCH // 2
    xq = nc.dram_tensor("xq", [NSC, 128, 2, IB, HA], bf, kind="ExternalInput").ap()
    xk = nc.dram_tensor("xk", [NSC, 128, 2, IB, HA], bf, kind="ExternalInput").ap()
    xv = nc.dram_tensor("xv", [NSC, 128, 4, IB, 128], bf, kind="ExternalInput").ap()
    wq = nc.dram_tensor("wq", [2, 128, IB, 128], bf, kind="ExternalInput").ap()
    wk = nc.dram_tensor("wk", [2, 128, IB, 128], bf, kind="ExternalInput").ap()
    wv = nc.dram_tensor("wv", [128, IB, JS], bf, kind="ExternalInput").ap()
    wo = nc.dram_tensor("wo", [128, 2, D], bf, kind="ExternalInput").ap()
    bqd = nc.dram_tensor("bq", [2, 128], f32, kind="ExternalInput").ap()
    bkd = nc.dram_tensor("bk", [2, 128], f32, kind="ExternalInput").ap()
    triu2 = nc.dram_tensor("triu2", [128, 256], f32, kind="ExternalInput").ap()
    ident = nc.dram_tensor("ident", [128, 128], bf, kind="ExternalInput").ap()
    po = nc.dram_tensor("po", [D, S], bf, kind="ExternalOutput").ap()

    def mm(out, lhsT, rhs, **kw):
        nc.tensor.matmul(out, lhsT, rhs, **kw)

    with tile.TileContext(nc) as tc:
        with (
            tc.tile_pool(name="persist", bufs=1) as pp,
            tc.tile_pool(name="xin", bufs=3) as xpool,
            tc.tile_pool(name="work", bufs=4) as wk_pool,
            tc.tile_pool(name="outp", bufs=6) as opool,
            tc.tile_pool(name="psA", bufs=3, space="PSUM") as psA,
            tc.tile_pool(name="psB", bufs=5, space="PSUM") as psB,
        ):
            # ---- persistent tiles -------------------------------------------
            wq_sb = pp.tile([128, 2, IB, 128], bf, tag="wq", name="wq_sb")
            wk_sb = pp.tile([128, 2, IB, 128], bf, tag="wk", name="wk_sb")
            wv_sb = pp.tile([128, IB, JS], bf, tag="wv", name="wv_sb")
            wo_sb = pp.tile([128, 2, D], bf, tag="wo", name="wo_sb")
            bq_sb = pp.tile([128, 2], f32, tag="bq", name="bq_sb")
            bk_sb = pp.tile([128, 2], f32, tag="bk", name="bk_sb")
            triu_sb = pp.tile([128, 256], f32, tag="triu", name="triu_sb")
            ident_sb = pp.tile([128, 128], bf, tag="ident", name="ident_sb")

            # q in per-head zero-padded tiles: head hh lives on partitions
            # hh*64..hh*64+64, the other 64 partitions stay zero, so score
            # and Q@S matmuls use a full-128 stationary (shared kT weight
            # load + FWL) without changing the math.  Both heads of a jb
            # share one [128, 2, S] tile so the chunk's scores are a single
            # 256-column matmul.
            qTz = [pp.tile([128, 2, S], bf, tag=f"qTz{jb}", name=f"qTz{jb}")
                   for jb in range(2)]
            kT_sb = [pp.tile([128, S], bf, tag=f"kT{jb}", name=f"kT{jb}") for jb in range(2)]
            aT_c = pp.tile([128, 2, S], bf, tag="aTc", name="aTc")
            vaug_sb = [pp.tile([128, HPC * JAUG], bf, tag=f"vaug{i}", name=f"vaug{i}")
                       for i in range(NCH)]
            # two heads per tile: head h at partitions (h%2)*64 .. +64
            saug_sb = [pp.tile([128, JAUG], f32, tag=f"saug{jb}", name=f"saug{jb}")
                       for jb in range(2)]
            saug_bf = [pp.tile([128, JAUG], bf, tag=f"saugb{jb}", name=f"saugb{jb}")
                       for jb in range(2)]
            attn2_sb = [pp.tile([128, 2 * DEPTH], bf, tag=f"attn2{jb}", name=f"attn2{jb}")
                        for jb in range(2)]

            # zero the unused head half of each qTz tile once (vector is idle
            # during the startup DMA window)
            for jb in range(2):
                for hh in range(2):
                    z0 = (1 - hh) * 64
                    nc.vector.memset(qTz[jb][z0:z0 + 64, hh, :], 0.0)

            # ---- initial loads: q path first so compute starts ASAP ---------
            x_tiles = {}

            def load_x(sc):
                xq_t = xpool.tile([128, 2, IB, HA], bf, tag="xq")
                xk_t = xpool.tile([128, 2, IB, HA], bf, tag="xk")
                xv_t = xpool.tile([128, 4, IB, 128], bf, tag="xv")
                nc.sync.dma_start(xq_t[:], xq[sc])
                nc.scalar.dma_start(xk_t[:], xk[sc])
                nc.sync.dma_start(xv_t[:], xv[sc])
                x_tiles[sc] = (xq_t, xk_t, xv_t)

            # chunk-0 inputs arrive in halves/quarters so the projection
            # matmuls can start as soon as the first slice lands.  sync queue
            # carries the bulky x tensors in consumption order; the scalar
            # queue carries weights so both drain in parallel.
            xq_t0 = xpool.tile([128, 2, IB, HA], bf, tag="xq")
            xk_t0 = xpool.tile([128, 2, IB, HA], bf, tag="xk")
            xv_t0 = xpool.tile([128, 4, IB, 128], bf, tag="xv")
            xq_t1 = xpool.tile([128, 2, IB, HA], bf, tag="xq")
            xk_t1 = xpool.tile([128, 2, IB, HA], bf, tag="xk")
            xv_t1 = xpool.tile([128, 4, IB, 128], bf, tag="xv")
            nc.sync.dma_start(xq_t0[:, 0], xq[0][:, 0])
            nc.scalar.dma_start(wq_sb[:, 0], wq[0])
            nc.scalar.dma_start(bq_sb[:], bqd.rearrange("jb p -> p jb"))
            nc.scalar.dma_start(bk_sb[:], bkd.rearrange("jb p -> p jb"))
            nc.scalar.dma_start(wq_sb[:, 1], wq[1])
            nc.sync.dma_start(xq_t0[:, 1], xq[0][:, 1])
            nc.scalar.dma_start(wk_sb[:, 0], wk[0])
            nc.scalar.dma_start(triu_sb[:], triu2)
            nc.scalar.dma_start(ident_sb[:], ident)
            nc.scalar.dma_start(wk_sb[:, 1], wk[1])
            nc.sync.dma_start(xk_t0[:, 0], xk[0][:, 0])
            nc.sync.dma_start(xk_t0[:, 1], xk[0][:, 1])
            nc.scalar.dma_start(wv_sb[:], wv)
            for ss in range(SCH // 128):
                nc.sync.dma_start(xv_t0[:, ss], xv[0][:, ss])
            # group-1 x interleaved across both queues by first-use time;
            # wo is not needed until the first p3 unit (group 1)
            nc.scalar.dma_start(xk_t1[:], xk[1])
            nc.sync.dma_start(xq_t1[:], xq[1])
            nc.scalar.dma_start(wo_sb[:], wo)
            nc.sync.dma_start(xv_t1[:], xv[1])
            x_tiles[0] = (xq_t0, xk_t0, xv_t0)
            x_tiles[1] = (xq_t1, xk_t1, xv_t1)

            # ---- pumpable work units (one PSUM group each) ------------------
            def write_q(jb, s0, ncols, ps):
                for hh in range(2):
                    r0 = hh * 64
                    nc.scalar.activation(
                        qTz[jb][r0:r0 + 64, hh, s0:s0 + ncols],
                        ps[r0:r0 + 64, 0:ncols],
                        AF.Identity if SIM_NO_PRELU else AF.Prelu,
                        bias=bq_sb[r0:r0 + 64, jb:jb + 1], scale=1.0, alpha=0.1)

            def unit_qk(which, sc, jb):
                s0 = sc * SCH
                x_t = x_tiles[sc][0 if which == "q" else 1]
                w_sb = wq_sb if which == "q" else wk_sb
                ps = psA.tile([128, SCH], f32, tag="A")
                for ib in range(IB):
                    mm(ps[:], w_sb[:, jb, ib, :],
                       x_t[:, :, ib, :],
                       start=(ib == 0), stop=(ib == IB - 1))
                if which == "q":
                    write_q(jb, s0, SCH, ps)
                else:
                    nc.scalar.activation(
                        kT_sb[jb][:, s0:s0 + SCH], ps[:],
                        AF.Identity if SIM_NO_PRELU else AF.Prelu,
                        bias=bk_sb[:, jb:jb + 1], scale=1.0, alpha=0.1)

            def unit_v(sc, ss):
                x_t = x_tiles[sc][2]
                ps = psA.tile([128, JS], f32, tag="A")
                for ib in range(IB):
                    mm(ps[:], x_t[:, ss, ib, :],
                       wv_sb[:, ib, :],
                       start=(ib == 0), stop=(ib == IB - 1))
                vt = vaug_sb[sc * (SCH // 128) + ss]
                vt_r = vt[:].rearrange("p (h e) -> p h e", h=HPC)
                nc.scalar.activation(
                    vt_r[:, :, 0:DEPTH],
                    ps[:].rearrange("p (h e) -> p h e", h=HPC), AF.Copy)
                nc.gpsimd.memset(vt_r[:, :, DEPTH:JAUG], 1.0)

            def unit_qk_half(which, sc, jb, half):
                s0 = sc * SCH + half * (SCH // 2)
                x_t = x_tiles[sc][0 if which == "q" else 1]
                w_sb = wq_sb if which == "q" else wk_sb
                ps = psA.tile([128, SCH // 2], f32, tag="A")
                for ib in range(IB):
                    mm(ps[:], w_sb[:, jb, ib, :],
                       x_t[:, half, ib, :],
                       start=(ib == 0), stop=(ib == IB - 1))
                if which == "q":
                    write_q(jb, s0, SCH // 2, ps)
                else:
                    nc.scalar.activation(
                        kT_sb[jb][:, s0:s0 + SCH // 2], ps[:],
                        AF.Identity if SIM_NO_PRELU else AF.Prelu,
                        bias=bk_sb[:, jb:jb + 1], scale=1.0, alpha=0.1)

            def proj_units(sc, include_v=True):
                u = []
                for jb in range(2):
                    u.append(lambda jb=jb: unit_qk("q", sc, jb))
                    u.append(lambda jb=jb: unit_qk("k", sc, jb))
                if include_v:
                    for ss in range(SCH // 128):
                        u.append(lambda ss=ss: unit_v(sc, ss))
                return u

            def v_units(sc):
                return [lambda ss=ss: unit_v(sc, ss)
                        for ss in range(SCH // 128)]

            def proj_units0_qk():
                # chunk-0 variant: q/k at half-chunk granularity, ordered to
                # match the staggered arrival of the split input DMAs
                u = []
                for half in range(2):
                    for which in ("q", "k"):
                        for jb in range(2):
                            u.append(lambda w=which, jb=jb, h=half:
                                     unit_qk_half(w, 0, jb, h))
                return u

            po_r = po.rearrange("(ob p) s -> ob p s", p=128)

            def unit_p3(o0, osc, ob, alt=False):
                # alt: after the last attention chunk the psB banks are free,
                # so tail units alternate pools and the PE never waits on a
                # psum WAR behind the trailing copies
                if alt and ob % 2 == 1:
                    ps = psB.tile([128, OSC], f32, tag="B")
                else:
                    ps = psA.tile([128, OSC], f32, tag="A")
                for jb in range(2):
                    mm(ps[:, 0:osc], wo_sb[:, jb, ob * 128:(ob + 1) * 128],
                       aT_c[:, jb, o0:o0 + osc],
                       start=(jb == 0), stop=(jb == 1))
                ot = opool.tile([128, OSC], bf, tag="ot")
                if ob % 2 == 0:
                    nc.vector.tensor_copy(ot[:, 0:osc], ps[:, 0:osc])
                else:
                    nc.scalar.activation(ot[:, 0:osc], ps[:, 0:osc], AF.Copy)
                q_eng = nc.sync if ob % 2 == 0 else nc.scalar
                q_eng.dma_start(po_r[ob, :, o0:o0 + osc], ot[:, 0:osc])

            def p3_units(o0, osc=OSC, alt=False):
                return [lambda ob=ob: unit_p3(o0, osc, ob, alt)
                        for ob in range(D // 128)]

            pending = []
            deferred_tt = []

            def pump():
                if pending:
                    u = pending.pop(0)
                    if u is not None:
                        u()

            def pump2():
                # two units back-to-back: the second unit's first weight
                # load hides under the first unit's matmul stream, so only
                # one small->big LDWEIGHTS exposure is paid per pump point
                pump()
                pump()

            def flush_tt():
                while deferred_tt:
                    deferred_tt.pop(0)()

            # ---- attention chunk (pumps a work unit between stages) ---------
            def chunk(ci):
                scol = ci * C
                if ci > 0:
                    nc.vector.tensor_copy(saug_bf[0][:], saug_sb[0][:])
                    nc.gpsimd.tensor_copy(saug_bf[1][:], saug_sb[1][:])
                # previous chunk's attn transposes go first: their input
                # (attn2_sb) settled a full chunk ago, and emitting them
                # before this chunk's DVE work keeps their semaphore waits
                # trivially satisfied (no PE stall behind the DVE backlog)
                flush_tt()

                # stage 1: K transposes (both heads in one op) + scores.
                # kT chunk is the stationary for ONE 256-col score matmul
                # per jb (rhs = both zero-padded per-head q chunks); both
                # jb's scores share a single PSUM bank (the second matmul's
                # start=False region still has has_written clear, so it
                # overwrites rather than accumulates).
                kp2 = psB.tile([128, 2, 2 * DEPTH], bf, tag="B")
                for jb in range(2):
                    nc.tensor.transpose(kp2[:, jb, :],
                                        kT_sb[jb][:, scol:scol + C],
                                        ident_sb[:])
                knat2 = wk_pool.tile([128, 2, 2 * DEPTH], bf, tag="knat")
                nc.vector.tensor_copy(knat2[:], kp2[:])
                at2 = psA.tile([128, 2, 2 * C], f32, tag="A")
                atm = []
                for jb in range(2):
                    mm(at2[:, jb, :], kT_sb[jb][:, scol:scol + C],
                       qTz[jb][:, :, scol:scol + C],
                       start=(jb == 0), stop=(jb == 1),
                       skip_group_check=True)
                for jb in range(2):
                    am = wk_pool.tile([128, 2 * C], bf, tag="atm")
                    nc.vector.tensor_tensor(am[:], at2[:, jb, :],
                                            triu_sb[:], op=Alu.mult)
                    atm.append(am)
                if ci == 0:
                    pump()
                else:
                    pump2()

                # stage 2: numerators + attn, per jb
                for jb in range(2):
                    for hh in range(2):
                        h = jb * 2 + hh
                        vt = vaug_sb[ci][:, h * JAUG:(h + 1) * JAUG]
                        nump = psB.tile([128, JAUG], f32, tag="B")
                        mm(nump[:], atm[jb][:, hh * C:(hh + 1) * C], vt,
                           start=True, stop=(ci == 0))
                        if ci > 0:
                            mm(nump[:], qTz[jb][:, hh, scol:scol + C],
                               saug_bf[jb][:],
                               start=False, stop=True)
                        recip = wk_pool.tile([128, 1], f32, tag="recip")
                        nc.vector.reciprocal(recip[:], nump[:, DEPTH:JAUG])
                        dstap = attn2_sb[jb][:, hh * DEPTH:(hh + 1) * DEPTH]
                        if jb == 0:
                            nc.vector.tensor_scalar_mul(
                                dstap, nump[:, 0:DEPTH], recip[:])
                        else:
                            nc.scalar.activation(dstap, nump[:, 0:DEPTH],
                                                 AF.Copy, scale=recip[:])
                    if ci == 0:
                        pump()
                if ci > 0:
                    pump2()

                # stage 3: state update S_aug += K^T V_aug (col-tiled pair
                # per jb runs concurrently on disjoint PE column groups;
                # both jb share one PSUM bank via has_written semantics)
                if ci < NCH - 1:
                    d_ps2 = psB.tile([128, 2, JAUG], f32, tag="B")
                    for jb in range(2):
                        for hh in range(2):
                            jo = hh * DEPTH
                            h = jb * 2 + hh
                            vt = vaug_sb[ci][:, h * JAUG:(h + 1) * JAUG]
                            mm(d_ps2[jo:jo + DEPTH, jb, :],
                               knat2[:, jb, jo:jo + DEPTH],
                               vt, start=(jb == 0), stop=(jb == 1),
                               skip_group_check=True)
                    for jb in range(2):
                        if ci == 0:
                            nc.vector.tensor_copy(saug_sb[jb][:],
                                                  d_ps2[:, jb, :])
                        else:
                            nc.vector.tensor_add(saug_sb[jb][:],
                                                 saug_sb[jb][:],
                                                 d_ps2[:, jb, :])

                # stage 4: transpose attn -> aT columns (deferred into the
                # next chunk's stream so the attn-copy chain has cover)
                def tt(scol=scol):
                    at2_ps = psB.tile([128, 2, C], bf, tag="B")
                    for jb in range(2):
                        nc.tensor.transpose(at2_ps[:, jb, :], attn2_sb[jb][:],
                                            ident_sb[:])
                    nc.scalar.activation(aT_c[:, :, scol:scol + C],
                                         at2_ps[:], AF.Copy)
                deferred_tt.append(tt)
                if ci == 0:
                    pump()

            # ---- schedule ---------------------------------------------------
            # Each group pumps the NEXT group's projections + the PREVIOUS
            # group's output projection between its attention stages.  The
            # last group's V projection and the last p3 batch are held back
            # so the tail (which otherwise has only small matmuls) keeps the
            # PE clock gate open; the final 512 columns run as one dense
            # block after the final chunk.
            for u in proj_units0_qk():
                u()
            for g in range(NSC):
                if g + 2 < NSC:
                    load_x(g + 2)
                pending = []
                if g == 0:
                    pending = v_units(0) + proj_units(1)
                elif g == NSC - 1:
                    vs = v_units(g)
                    ps3 = p3_units((g - 1) * OSC)
                    pending = [vs[0], vs[1], ps3[0], vs[2], ps3[1], vs[3]]
                    pending += ps3[2:]
                    # first half of the last group's output projection: its
                    # aT columns (chunks 12-13) are flushed by chunk 14's
                    # stage 1, and these units sit deep enough in `pending`
                    # that they pump no earlier than chunk 15
                    pending += p3_units(g * OSC, osc=OSC // 2)
                else:
                    a = proj_units(g + 1, include_v=(g + 1 < NSC - 1))
                    b = p3_units((g - 1) * OSC)
                    while a or b:
                        if a:
                            pending.append(a.pop(0))
                        if b:
                            pending.append(b.pop(0))
                for t in range(CPO):
                    chunk(CPO * g + t)
                while pending:
                    pump()
            flush_tt()
            # tail: the last 256 output columns as one dense block (the
            # first 256 were pumped through chunk 15; copies and stores
            # trail on vector/scalar + both queues)
            for u in p3_units((NSC - 1) * OSC + OSC // 2, osc=OSC // 2,
                              alt=True):
                u()

    nc.compile()
    return nc


def _get_prog():
    global _PROG
    if _PROG is None:
        _PROG = _build()
    return _PROG


def kernel(q, k, v, query_mask, key_mask, value_mask,
           Wq, bq, Wk, bk, Wv, bv, Wo, bo):
    global LAST_RESULTS
    from concourse import bass_utils

    q = np.asarray(q, np.float32)
    k = np.asarray(k, np.float32)
    v = np.asarray(v, np.float32)
    qm = q * np.asarray(query_mask, np.float32)
    km = k * np.asarray(key_mask, np.float32)
    vm = v * np.asarray(value_mask, np.float32)
    Wq = np.asarray(Wq, np.float32)
    Wk = np.asarray(Wk, np.float32)
    Wv = np.asarray(Wv, np.float32)
    Wo = np.asarray(Wo, np.float32)
    bq = np.asarray(bq, np.float32)
    bk = np.asarray(bk, np.float32)
    bv = np.asarray(bv, np.float32)
    bo = np.asarray(bo, np.float32)
    assert not np.any(bv), "kernel assumes bv == 0 (true for this problem)"

    nc = _get_prog()

    triu1 = np.triu(np.ones((128, 128), np.float32))
    triu2 = np.concatenate([triu1, triu1], axis=1)
    ident = np.eye(128, dtype=np.float32).astype(BF16)

    HA = SCH // 2

    def tile_x2(a):  # [S, D] -> [NSC, 128, 2, IB, HA] (half-major)
        return np.ascontiguousarray(
            a.reshape(NSC, 2, HA, IB, 128).transpose(0, 4, 1, 3, 2)).astype(BF16)

    def tile_x4(a):  # [S, D] -> [NSC, 128, 4, IB, 128] (quarter-major)
        return np.ascontiguousarray(
            a.reshape(NSC, 4, 128, IB, 128).transpose(0, 4, 1, 3, 2)).astype(BF16)

    def tile_w(w):  # w: [D, JS] -> [128, IB, JS]
        return w.reshape(IB, 128, JS).transpose(1, 0, 2).astype(BF16)

    def tile_w2(w):  # w: [D, JS] -> [2, 128, IB, 128] (jb-major)
        return np.ascontiguousarray(
            w.reshape(IB, 128, 2, 128).transpose(2, 1, 0, 3)).astype(BF16)

    xqs = [tile_x2(qm[b]) for b in range(B)]
    xks = [tile_x2(km[b]) for b in range(B)]
    xvs = [tile_x4(vm[b]) for b in range(B)]

    in_maps = []
    for c in range(N_CORES):
        b, g = divmod(c, HPC)
        js = slice(g * JS, (g + 1) * JS)
        in_maps.append({
            "xq": xqs[b], "xk": xks[b], "xv": xvs[b],
            "wq": tile_w2(Wq[:, js]),
            "wk": tile_w2(Wk[:, js]),
            "wv": tile_w(Wv[:, js]),
            "wo": Wo[js, :].reshape(2, 128, D).transpose(1, 0, 2).astype(BF16),
            "bq": np.ascontiguousarray(bq[js].reshape(2, 128)),
            "bk": np.ascontiguousarray(bk[js].reshape(2, 128)),
            "triu2": triu2, "ident": ident,
        })

    res = bass_utils.run_bass_kernel_spmd(
        nc, in_maps, core_ids=list(range(N_CORES)),
        trace=TRACE, trace_cores=TRACE_CORES)
    LAST_RESULTS = res

    out = np.zeros((B, S, D), np.float32)
    for c in range(N_CORES):
        out[c // HPC] += res.results[c]["po"].astype(np.float32).T
    out += bo
    return out


# revision 49
# speedup vs baseline: 1.1681x; 1.1681x over previous
"""Trainium2 Bass kernel for nn_MultiHeadAttention_KT (causal linear attention).

Math (per batch b):
  q' = leaky((q*qm) @ Wq + bq); k' = leaky((k*km) @ Wk + bk); v' = (v*vm) @ Wv
  per head h (DEPTH=64):   S_t = sum_{s<=t} k_s v_s^T ; z_t = sum_{s<=t} k_s
                           attn_t = (q_t @ S_t) / (q_t . z_t)
  out = concat_heads(attn) @ Wo + bo
Sharding: 8 cores = 2 batches x 4 head-groups (4 heads / 256 cols each).
Host transposes + bf16-casts inputs (xq = (q*qm)^T etc.); host sums the 4
partial output projections per batch (po is this core's heads' Wo slice).

All matmul operands are bf16 (1 cyc/row on PE + fast weight load); PSUM
accumulation stays f32.  Chunked linear attention (chunk C=128):
  AT   = K Q^T (per chunk, [s,t] layout)      masked with triu (s<=t)
  num  = ATm^T V_aug + Q S_aug                (V_aug = [V | 1], S_aug = [S | z])
  attn = num[:, :64] * (1/num[:, 64])
  S_aug += K_chunk^T V_aug                    (delta matmul; f32 master state
                                               on DVE + bf16 mirror)

q' is stored in per-head zero-padded tiles (qTz[jb][hh], [128, S] with the
other head's 64 partitions zeroed) so every attention matmul has a 128-tall
stationary: scores share one kT LDWEIGHTS for both heads and all stationaries
qualify for fast weight load, which keeps the small-matmul issue rate near
the stream rate.  Projection / output-projection units are pumped one at a
time between attention stages so the PE HAM clock gate stays open; the last
group's V projection and the previous group's output projection are held
back to feed the tail, where no other big matmuls remain.
"""

import os
import sys

sys.path.insert(0, "/opt/trn_rl_repo")

import ml_dtypes
import numpy as np

BF16 = np.dtype(ml_dtypes.bfloat16)

B, S, D, H = 2, 2048, 1024, 16
DEPTH = 64
N_CORES = 8
HPC = 4                 # heads per core
JS = HPC * DEPTH        # 256 projected columns per core
C = 128                 # attention chunk length
NCH = S // C            # 16 chunks
IB = D // 128           # 8 contraction blocks
SCH = 512               # projection s-chunk
NSC = S // SCH          # 4 projection chunks
JAUG = DEPTH + 1        # 65 (V augmented with ones column)
OSC = 512               # output projection s-chunk
CPO = OSC // C          # attention chunks per output chunk

MM_DTYPE = "bf16"       # informational (printed by test harness)
SIM_NO_PRELU = os.environ.get("KT_SIM_NO_PRELU") == "1"  # CoreSim lacks Prelu
TRACE = False           # set True from test harness to capture NTFF profile
TRACE_CORES = None
LAST_RESULTS = None     # BassKernelResults of the last kernel() call

_PROG = None


def _build():
    import concourse.bacc as bacc
    import concourse.mybir as mybir
    import concourse.tile as tile

    dt = mybir.dt
    f32 = dt.float32
    bf = dt.bfloat16
    AF = mybir.ActivationFunctionType
    Alu = mybir.AluOpType

    nc = bacc.Bacc("TRN2", target_bir_lowering=False, debug=False,
                   num_devices=N_CORES)

    # host pre-tiles x and weights so every DMA is per-partition contiguous.
    # xq/xk are half-major and xv quarter-major so the startup split loads
    # are single 4KB/2KB descriptor rows instead of 512B fragments (the DMA
    # engines are descriptor-rate bound at kernel start).
    HA = SCH // 2
    xq = nc.dram_tensor("xq", [NSC, 128, 2, IB, HA], bf, kind="ExternalInput").ap()
    xk = nc.dram_tensor("xk", [NSC, 128, 2, IB, HA], bf, kind="ExternalInput").ap()
    xv = nc.dram_tensor("xv", [NSC, 128, 4, IB, 128], bf, kind="ExternalInput").ap()
    wq = nc.dram_tensor("wq", [2, 128, IB, 128], bf, kind="ExternalInput").ap()
    wk = nc.dram_tensor("wk", [2, 128, IB, 128], bf, kind="ExternalInput").ap()
    wv = nc.dram_tensor("wv", [128, IB, JS], bf, kind="ExternalInput").ap()
    wo = nc.dram_tensor("wo", [128, 2, D], bf, kind="ExternalInput").ap()
    bqd = nc.dram_tensor("bq", [2, 128], f32, kind="ExternalInput").ap()
    bkd = nc.dram_tensor("bk", [2, 128], f32, kind="ExternalInput").ap()
    triu2 = nc.dram_tensor("triu2", [128, 256], f32, kind="ExternalInput").ap()
    ident = nc.dram_tensor("ident", [128, 128], bf, kind="ExternalInput").ap()
    po = nc.dram_tensor("po", [D, S], bf, kind="ExternalOutput").ap()

    def mm(out, lhsT, rhs, **kw):
        nc.tensor.matmul(out, lhsT, rhs, **kw)

    with tile.TileContext(nc) as tc:
        with (
            tc.tile_pool(name="persist", bufs=1) as pp,
            tc.tile_pool(name="xin", bufs=3) as xpool,
            tc.tile_pool(name="work", bufs=4) as wk_pool,
            tc.tile_pool(name="outp", bufs=6) as opool,
            tc.tile_pool(name="psA", bufs=3, space="PSUM") as psA,
            tc.tile_pool(name="psB", bufs=5, space="PSUM") as psB,
        ):
            # ---- persistent tiles -------------------------------------------
            wq_sb = pp.tile([128, 2, IB, 128], bf, tag="wq", name="wq_sb")
            wk_sb = pp.tile([128, 2, IB, 128], bf, tag="wk", name="wk_sb")
            wv_sb = pp.tile([128, IB, JS], bf, tag="wv", name="wv_sb")
            wo_sb = pp.tile([128, 2, D], bf, tag="wo", name="wo_sb")
            bq_sb = pp.tile([128, 2], f32, tag="bq", name="bq_sb")
            bk_sb = pp.tile([128, 2], f32, tag="bk", name="bk_sb")
            triu_sb = pp.tile([128, 256], f32, tag="triu", name="triu_sb")
            ident_sb = pp.tile([128, 128], bf, tag="ident", name="ident_sb")

            # q in per-head zero-padded tiles: head hh lives on partitions
            # hh*64..hh*64+64, the other 64 partitions stay zero, so score
            # and Q@S matmuls use a full-128 stationary (shared kT weight
            # load + FWL) without changing the math.  Both heads of a jb
            # share one [128, 2, S] tile so the chunk's scores are a single
            # 256-column matmul.
            qTz = [pp.tile([128, 2, S], bf, tag=f"qTz{jb}", name=f"qTz{jb}")
                   for jb in range(2)]
            kT_sb = [pp.tile([128, S], bf, tag=f"kT{jb}", name=f"kT{jb}") for jb in range(2)]
            aT_c = pp.tile([128, 2, S], bf, tag="aTc", name="aTc")
            vaug_sb = [pp.tile([128, HPC * JAUG], bf, tag=f"vaug{i}", name=f"vaug{i}")
                       for i in range(NCH)]
            # two heads per tile: head h at partitions (h%2)*64 .. +64
            saug_sb = [pp.tile([128, JAUG], f32, tag=f"saug{jb}", name=f"saug{jb}")
                       for jb in range(2)]
            saug_bf = [pp.tile([128, JAUG], bf, tag=f"saugb{jb}", name=f"saugb{jb}")
                       for jb in range(2)]
            attn2_sb = [pp.tile([128, 2 * DEPTH], bf, tag=f"attn2{jb}", name=f"attn2{jb}")
                        for jb in range(2)]

            # zero the unused head half of each qTz tile once (vector is idle
            # during the startup DMA window)
            for jb in range(2):
                for hh in range(2):
                    z0 = (1 - hh) * 64
                    nc.vector.memset(qTz[jb][z0:z0 + 64, hh, :], 0.0)

            # ---- initial loads: q path first so compute starts ASAP ---------
            x_tiles = {}

            def load_x(sc):
                xq_t = xpool.tile([128, 2, IB, HA], bf, tag="xq")
                xk_t = xpool.tile([128, 2, IB, HA], bf, tag="xk")
                xv_t = xpool.tile([128, 4, IB, 128], bf, tag="xv")
                nc.sync.dma_start(xq_t[:], xq[sc])
                nc.scalar.dma_start(xk_t[:], xk[sc])
                nc.sync.dma_start(xv_t[:], xv[sc])
                x_tiles[sc] = (xq_t, xk_t, xv_t)

            # chunk-0 inputs arrive in halves/quarters so the projection
            # matmuls can start as soon as the first slice lands.  sync queue
            # carries the bulky x tensors in consumption order; the scalar
            # queue carries weights so both drain in parallel.
            xq_t0 = xpool.tile([128, 2, IB, HA], bf, tag="xq")
            xk_t0 = xpool.tile([128, 2, IB, HA], bf, tag="xk")
            xv_t0 = xpool.tile([128, 4, IB, 128], bf, tag="xv")
            xq_t1 = xpool.tile([128, 2, IB, HA], bf, tag="xq")
            xk_t1 = xpool.tile([128, 2, IB, HA], bf, tag="xk")
            xv_t1 = xpool.tile([128, 4, IB, 128], bf, tag="xv")
            nc.sync.dma_start(xq_t0[:, 0], xq[0][:, 0])
            nc.scalar.dma_start(wq_sb[:, 0], wq[0])
            nc.scalar.dma_start(bq_sb[:], bqd.rearrange("jb p -> p jb"))
            nc.scalar.dma_start(bk_sb[:], bkd.rearrange("jb p -> p jb"))
            nc.scalar.dma_start(wq_sb[:, 1], wq[1])
            nc.sync.dma_start(xq_t0[:, 1], xq[0][:, 1])
            nc.scalar.dma_start(wk_sb[:, 0], wk[0])
            nc.scalar.dma_start(triu_sb[:], triu2)
            nc.scalar.dma_start(ident_sb[:], ident)
            nc.scalar.dma_start(wk_sb[:, 1], wk[1])
            nc.sync.dma_start(xk_t0[:, 0], xk[0][:, 0])
            nc.sync.dma_start(xk_t0[:, 1], xk[0][:, 1])
            nc.scalar.dma_start(wv_sb[:], wv)
            for ss in range(SCH // 128):
                nc.sync.dma_start(xv_t0[:, ss], xv[0][:, ss])
            # group-1 x interleaved across both queues by first-use time;
            # wo is not needed until the first p3 unit (group 1)
            nc.scalar.dma_start(xk_t1[:], xk[1])
            nc.sync.dma_start(xq_t1[:], xq[1])
            nc.scalar.dma_start(wo_sb[:], wo)
            nc.sync.dma_start(xv_t1[:], xv[1])
            x_tiles[0] = (xq_t0, xk_t0, xv_t0)
            x_tiles[1] = (xq_t1, xk_t1, xv_t1)

            # ---- pumpable work units (one PSUM group each) ------------------
            def write_q(jb, s0, ncols, ps):
                for hh in range(2):
                    r0 = hh * 64
                    nc.scalar.activation(
                        qTz[jb][r0:r0 + 64, hh, s0:s0 + ncols],
                        ps[r0:r0 + 64, 0:ncols],
                        AF.Identity if SIM_NO_PRELU else AF.Prelu,
                        bias=bq_sb[r0:r0 + 64, jb:jb + 1], scale=1.0, alpha=0.1)

            def unit_qk(which, sc, jb):
                s0 = sc * SCH
                x_t = x_tiles[sc][0 if which == "q" else 1]
                w_sb = wq_sb if which == "q" else wk_sb
                ps = psA.tile([128, SCH], f32, tag="A")
                for ib in range(IB):
                    mm(ps[:], w_sb[:, jb, ib, :],
                       x_t[:, :, ib, :],
                       start=(ib == 0), stop=(ib == IB - 1))
                if which == "q":
                    write_q(jb, s0, SCH, ps)
                else:
                    nc.scalar.activation(
                        kT_sb[jb][:, s0:s0 + SCH], ps[:],
                        AF.Identity if SIM_NO_PRELU else AF.Prelu,
                        bias=bk_sb[:, jb:jb + 1], scale=1.0, alpha=0.1)

            def unit_v(sc, ss):
                x_t = x_tiles[sc][2]
                ps = psA.tile([128, JS], f32, tag="A")
                for ib in range(IB):
                    mm(ps[:], x_t[:, ss, ib, :],
                       wv_sb[:, ib, :],
                       start=(ib == 0), stop=(ib == IB - 1))
                vt = vaug_sb[sc * (SCH // 128) + ss]
                vt_r = vt[:].rearrange("p (h e) -> p h e", h=HPC)
                nc.scalar.activation(
                    vt_r[:, :, 0:DEPTH],
                    ps[:].rearrange("p (h e) -> p h e", h=HPC), AF.Copy)
                nc.gpsimd.memset(vt_r[:, :, DEPTH:JAUG], 1.0)

            def unit_qk_half(which, sc, jb, half):
                s0 = sc * SCH + half * (SCH // 2)
                x_t = x_tiles[sc][0 if which == "q" else 1]
                w_sb = wq_sb if which == "q" else wk_sb
                ps = psA.tile([128, SCH // 2], f32, tag="A")
                for ib in range(IB):
                    mm(ps[:], w_sb[:, jb, ib, :],
                       x_t[:, half, ib, :],
                       start=(ib == 0), stop=(ib == IB - 1))
                if which == "q":
                    write_q(jb, s0, SCH // 2, ps)
                else:
                    nc.scalar.activation(
                        kT_sb[jb][:, s0:s0 + SCH // 2], ps[:],
                        AF.Identity if SIM_NO_PRELU else AF.Prelu,
                        bias=bk_sb[:, jb:jb + 1], scale=1.0, alpha=0.1)

            def proj_units(sc, include_v=True):
                u = []
                for jb in range(2):
                    u.append(lambda jb=jb: unit_qk("q", sc, jb))
                    u.append(lambda jb=jb: unit_qk("k", sc, jb))
                if include_v:
                    for ss in range(SCH // 128):
                        u.append(lambda ss=ss: unit_v(sc, ss))
                return u

            def v_units(sc):
                return [lambda ss=ss: unit_v(sc, ss)
                        for ss in range(SCH // 128)]

            def proj_units0_qk():
                # chunk-0 variant: q/k at half-chunk granularity, ordered to
                # match the staggered arrival of the split input DMAs
                u = []
                for half in range(2):
                    for which in ("q", "k"):
                        for jb in range(2):
                            u.append(lambda w=which, jb=jb, h=half:
                                     unit_qk_half(w, 0, jb, h))
                return u

            po_r = po.rearrange("(ob p) s -> ob p s", p=128)

            def unit_p3(o0, osc, ob, alt=False):
                # alt: after the last attention chunk the psB banks are free,
                # so tail units alternate pools and the PE never waits on a
                # psum WAR behind the trailing copies
                if alt and ob % 2 == 1:
                    ps = psB.tile([128, OSC], f32, tag="B")
                else:
                    ps = psA.tile([128, OSC], f32, tag="A")
                for jb in range(2):
                    mm(ps[:, 0:osc], wo_sb[:, jb, ob * 128:(ob + 1) * 128],
                       aT_c[:, jb, o0:o0 + osc],
                       start=(jb == 0), stop=(jb == 1))
                ot = opool.tile([128, OSC], bf, tag="ot")
                if ob % 2 == 0:
                    nc.vector.tensor_copy(ot[:, 0:osc], ps[:, 0:osc])
                else:
                    nc.scalar.activation(ot[:, 0:osc], ps[:, 0:osc], AF.Copy)
                q_eng = nc.sync if ob % 2 == 0 else nc.scalar
                q_eng.dma_start(po_r[ob, :, o0:o0 + osc], ot[:, 0:osc])

            def p3_units(o0, osc=OSC, alt=False):
                return [lambda ob=ob: unit_p3(o0, osc, ob, alt)
                        for ob in range(D // 128)]

            pending = []
            deferred_tt = []

            def pump():
                if pending:
                    u = pending.pop(0)
                    if u is not None:
                        u()

            def flush_tt():
                while deferred_tt:
                    deferred_tt.pop(0)()

            # ---- attention chunk (pumps a work unit between stages) ---------
            def chunk(ci):
                scol = ci * C
                if ci > 0:
                    nc.vector.tensor_copy(saug_bf[0][:], saug_sb[0][:])
                    nc.gpsimd.tensor_copy(saug_bf[1][:], saug_sb[1][:])
                # previous chunk's attn transposes go first: their input
                # (attn2_sb) settled a full chunk ago, and emitting them
                # before this chunk's DVE work keeps their semaphore waits
                # trivially satisfied (no PE stall behind the DVE backlog)
                flush_tt()

                # stage 1: K transposes (both heads in one op) + scores.
                # kT chunk is the stationary for ONE 256-col score matmul
                # per jb (rhs = both zero-padded per-head q chunks); both
                # jb's scores share a single PSUM bank (the second matmul's
                # start=False region still has has_written clear, so it
                # overwrites rather than accumulates).
                kp2 = psB.tile([128, 2, 2 * DEPTH], bf, tag="B")
                for jb in range(2):
                    nc.tensor.transpose(kp2[:, jb, :],
                                        kT_sb[jb][:, scol:scol + C],
                                        ident_sb[:])
                knat2 = wk_pool.tile([128, 2, 2 * DEPTH], bf, tag="knat")
                nc.vector.tensor_copy(knat2[:], kp2[:])
                at2 = psA.tile([128, 2, 2 * C], f32, tag="A")
                atm = []
                for jb in range(2):
                    mm(at2[:, jb, :], kT_sb[jb][:, scol:scol + C],
                       qTz[jb][:, :, scol:scol + C],
                       start=(jb == 0), stop=(jb == 1),
                       skip_group_check=True)
                for jb in range(2):
                    am = wk_pool.tile([128, 2 * C], bf, tag="atm")
                    nc.vector.tensor_tensor(am[:], at2[:, jb, :],
                                            triu_sb[:], op=Alu.mult)
                    atm.append(am)
                pump()

                # stage 2: numerators + attn, per jb
                for jb in range(2):
                    for hh in range(2):
                        h = jb * 2 + hh
                        vt = vaug_sb[ci][:, h * JAUG:(h + 1) * JAUG]
                        nump = psB.tile([128, JAUG], f32, tag="B")
                        mm(nump[:], atm[jb][:, hh * C:(hh + 1) * C], vt,
                           start=True, stop=(ci == 0))
                        if ci > 0:
                            mm(nump[:], qTz[jb][:, hh, scol:scol + C],
                               saug_bf[jb][:],
                               start=False, stop=True)
                        recip = wk_pool.tile([128, 1], f32, tag="recip")
                        nc.vector.reciprocal(recip[:], nump[:, DEPTH:JAUG])
                        dstap = attn2_sb[jb][:, hh * DEPTH:(hh + 1) * DEPTH]
                        if jb == 0:
                            nc.vector.tensor_scalar_mul(
                                dstap, nump[:, 0:DEPTH], recip[:])
                        else:
                            nc.scalar.activation(dstap, nump[:, 0:DEPTH],
                                                 AF.Copy, scale=recip[:])
                    pump()

                # stage 3: state update S_aug += K^T V_aug (col-tiled pair
                # per jb runs concurrently on disjoint PE column groups;
                # both jb share one PSUM bank via has_written semantics)
                if ci < NCH - 1:
                    d_ps2 = psB.tile([128, 2, JAUG], f32, tag="B")
                    for jb in range(2):
                        for hh in range(2):
                            jo = hh * DEPTH
                            h = jb * 2 + hh
                            vt = vaug_sb[ci][:, h * JAUG:(h + 1) * JAUG]
                            mm(d_ps2[jo:jo + DEPTH, jb, :],
                               knat2[:, jb, jo:jo + DEPTH],
                               vt, start=(jb == 0), stop=(jb == 1),
                               skip_group_check=True)
                    for jb in range(2):
                        if ci == 0:
                            nc.vector.tensor_copy(saug_sb[jb][:],
                                                  d_ps2[:, jb, :])
                        else:
                            nc.vector.tensor_add(saug_sb[jb][:],
                                                 saug_sb[jb][:],
                                                 d_ps2[:, jb, :])

                # stage 4: transpose attn -> aT columns (deferred into the
                # next chunk's stream so the attn-copy chain has cover)
                def tt(scol=scol):
                    at2_ps = psB.tile([128, 2, C], bf, tag="B")
                    for jb in range(2):
                        nc.tensor.transpose(at2_ps[:, jb, :], attn2_sb[jb][:],
                                            ident_sb[:])
                    nc.scalar.activation(aT_c[:, :, scol:scol + C],
                                         at2_ps[:], AF.Copy)
                deferred_tt.append(tt)
                pump()

            # ---- schedule ---------------------------------------------------
            # Each group pumps the NEXT group's projections + the PREVIOUS
            # group's output projection between its attention stages.  The
            # last group's V projection and the last p3 batch are held back
            # so the tail (which otherwise has only small matmuls) keeps the
            # PE clock gate open; the final 512 columns run as one dense
            # block after the final chunk.
            for u in proj_units0_qk():
                u()
            for g in range(NSC):
                if g + 2 < NSC:
                    load_x(g + 2)
                pending = []
                if g == 0:
                    pending = v_units(0) + proj_units(1)
                elif g == NSC - 1:
                    vs = v_units(g)
                    ps3 = p3_units((g - 1) * OSC)
                    pending = [vs[0], vs[1], ps3[0], vs[2], ps3[1], vs[3]]
                    pending += ps3[2:]
                    # first half of the last group's output projection: its
                    # aT columns (chunks 12-13) are flushed by chunk 14's
                    # stage 1, and these units sit deep enough in `pending`
                    # that they pump no earlier than chunk 15
                    pending += p3_units(g * OSC, osc=OSC // 2)
                else:
                    a = proj_units(g + 1, include_v=(g + 1 < NSC - 1))
                    b = p3_units((g - 1) * OSC)
                    while a or b:
                        if a:
                            pending.append(a.pop(0))
                        if b:
                            pending.append(b.pop(0))
                for t in range(CPO):
                    chunk(CPO * g + t)
                while pending:
                    pump()
            flush_tt()
            # tail: the last 256 output columns as one dense block (the
            # first 256 were pumped through chunk 15; copies and stores
            # trail on vector/scalar + both queues)
            for u in p3_units((NSC - 1) * OSC + OSC // 2, osc=OSC // 2,
                              alt=True):
                u()

    nc.compile()
    return nc


def _get_prog():
    global _PROG
    if _PROG is None:
        _PROG = _build()
    return _PROG


def kernel(q, k, v, query_mask, key_mask, value_mask,
           Wq, bq, Wk, bk, Wv, bv, Wo, bo):
    global LAST_RESULTS
    from concourse import bass_utils

    q = np.asarray(q, np.float32)
    k = np.asarray(k, np.float32)
    v = np.asarray(v, np.float32)
    qm = q * np.asarray(query_mask, np.float32)
    km = k * np.asarray(key_mask, np.float32)
    vm = v * np.asarray(value_mask, np.float32)
    Wq = np.asarray(Wq, np.float32)
    Wk = np.asarray(Wk, np.float32)
    Wv = np.asarray(Wv, np.float32)
    Wo = np.asarray(Wo, np.float32)
    bq = np.asarray(bq, np.float32)
    bk = np.asarray(bk, np.float32)
    bv = np.asarray(bv, np.float32)
    bo = np.asarray(bo, np.float32)
    assert not np.any(bv), "kernel assumes bv == 0 (true for this problem)"

    nc = _get_prog()

    triu1 = np.triu(np.ones((128, 128), np.float32))
    triu2 = np.concatenate([triu1, triu1], axis=1)
    ident = np.eye(128, dtype=np.float32).astype(BF16)

    HA = SCH // 2

    def tile_x2(a):  # [S, D] -> [NSC, 128, 2, IB, HA] (half-major)
        return np.ascontiguousarray(
            a.reshape(NSC, 2, HA, IB, 128).transpose(0, 4, 1, 3, 2)).astype(BF16)

    def tile_x4(a):  # [S, D] -> [NSC, 128, 4, IB, 128] (quarter-major)
        return np.ascontiguousarray(
            a.reshape(NSC, 4, 128, IB, 128).transpose(0, 4, 1, 3, 2)).astype(BF16)

    def tile_w(w):  # w: [D, JS] -> [128, IB, JS]
        return w.reshape(IB, 128, JS).transpose(1, 0, 2).astype(BF16)

    def tile_w2(w):  # w: [D, JS] -> [2, 128, IB, 128] (jb-major)
        return np.ascontiguousarray(
            w.reshape(IB, 128, 2, 128).transpose(2, 1, 0, 3)).astype(BF16)

    xqs = [tile_x2(qm[b]) for b in range(B)]
    xks = [tile_x2(km[b]) for b in range(B)]
    xvs = [tile_x4(vm[b]) for b in range(B)]

    in_maps = []
    for c in range(N_CORES):
        b, g = divmod(c, HPC)
        js = slice(g * JS, (g + 1) * JS)
        in_maps.append({
            "xq": xqs[b], "xk": xks[b], "xv": xvs[b],
            "wq": tile_w2(Wq[:, js]),
            "wk": tile_w2(Wk[:, js]),
            "wv": tile_w(Wv[:, js]),
            "wo": Wo[js, :].reshape(2, 128, D).transpose(1, 0, 2).astype(BF16),
            "bq": np.ascontiguousarray(bq[js].reshape(2, 128)),
            "bk": np.ascontiguousarray(bk[js].reshape(2, 128)),
            "triu2": triu2, "ident": ident,
        })

    res = bass_utils.run_bass_kernel_spmd(
        nc, in_maps, core_ids=list(range(N_CORES)),
        trace=TRACE, trace_cores=TRACE_CORES)
    LAST_RESULTS = res

    out = np.zeros((B, S, D), np.float32)
    for c in range(N_CORES):
        out[c // HPC] += res.results[c]["po"].astype(np.float32).T
    out += bo
    return out


# revision 52
# speedup vs baseline: 1.1899x; 1.0186x over previous
"""Trainium2 Bass kernel for nn_MultiHeadAttention_KT (causal linear attention).

Math (per batch b):
  q' = leaky((q*qm) @ Wq + bq); k' = leaky((k*km) @ Wk + bk); v' = (v*vm) @ Wv
  per head h (DEPTH=64):   S_t = sum_{s<=t} k_s v_s^T ; z_t = sum_{s<=t} k_s
                           attn_t = (q_t @ S_t) / (q_t . z_t)
  out = concat_heads(attn) @ Wo + bo
Sharding: 8 cores = 2 batches x 4 head-groups (4 heads / 256 cols each).
Host transposes + bf16-casts inputs (xq = (q*qm)^T etc.); host sums the 4
partial output projections per batch (po is this core's heads' Wo slice).

All matmul operands are bf16 (1 cyc/row on PE + fast weight load); PSUM
accumulation stays f32.  Chunked linear attention (chunk C=128):
  AT   = K Q^T (per chunk, [s,t] layout)      masked with triu (s<=t)
  num  = ATm^T V_aug + Q S_aug                (V_aug = [V | 1], S_aug = [S | z])
  attn = num[:, :64] * (1/num[:, 64])
  S_aug += K_chunk^T V_aug                    (delta matmul; f32 master state
                                               on DVE + bf16 mirror)

q' is stored in per-head zero-padded tiles (qTz[jb][hh], [128, S] with the
other head's 64 partitions zeroed) so every attention matmul has a 128-tall
stationary: scores share one kT LDWEIGHTS for both heads and all stationaries
qualify for fast weight load, which keeps the small-matmul issue rate near
the stream rate.  Projection / output-projection units are pumped one at a
time between attention stages so the PE HAM clock gate stays open; the last
group's V projection and the previous group's output projection are held
back to feed the tail, where no other big matmuls remain.
"""

import os
import sys

sys.path.insert(0, "/opt/trn_rl_repo")

import ml_dtypes
import numpy as np

BF16 = np.dtype(ml_dtypes.bfloat16)

B, S, D, H = 2, 2048, 1024, 16
DEPTH = 64
N_CORES = 8
HPC = 4                 # heads per core
JS = HPC * DEPTH        # 256 projected columns per core
C = 128                 # attention chunk length
NCH = S // C            # 16 chunks
IB = D // 128           # 8 contraction blocks
SCH = 512               # projection s-chunk
NSC = S // SCH          # 4 projection chunks
JAUG = DEPTH + 1        # 65 (V augmented with ones column)
OSC = 512               # output projection s-chunk
CPO = OSC // C          # attention chunks per output chunk

MM_DTYPE = "bf16"       # informational (printed by test harness)
SIM_NO_PRELU = os.environ.get("KT_SIM_NO_PRELU") == "1"  # CoreSim lacks Prelu
TRACE = False           # set True from test harness to capture NTFF profile
TRACE_CORES = None
LAST_RESULTS = None     # BassKernelResults of the last kernel() call

_PROG = None


def _build():
    import concourse.bacc as bacc
    import concourse.mybir as mybir
    import concourse.tile as tile

    dt = mybir.dt
    f32 = dt.float32
    bf = dt.bfloat16
    AF = mybir.ActivationFunctionType
    Alu = mybir.AluOpType

    nc = bacc.Bacc("TRN2", target_bir_lowering=False, debug=False,
                   num_devices=N_CORES)

    # host pre-tiles x and weights so every DMA is per-partition contiguous.
    # xq/xk are half-major and xv quarter-major so the startup split loads
    # are single 4KB/2KB descriptor rows instead of 512B fragments (the DMA
    # engines are descriptor-rate bound at kernel start).
    HA = SCH // 2
    xq = nc.dram_tensor("xq", [NSC, 128, 2, IB, HA], bf, kind="ExternalInput").ap()
    xk = nc.dram_tensor("xk", [NSC, 128, 2, IB, HA], bf, kind="ExternalInput").ap()
    xv = nc.dram_tensor("xv", [NSC, 128, 4, IB, 128], bf, kind="ExternalInput").ap()
    wq = nc.dram_tensor("wq", [2, 128, IB, 128], bf, kind="ExternalInput").ap()
    wk = nc.dram_tensor("wk", [2, 128, IB, 128], bf, kind="ExternalInput").ap()
    wv = nc.dram_tensor("wv", [128, IB, JS], bf, kind="ExternalInput").ap()
    wo = nc.dram_tensor("wo", [128, 2, D], bf, kind="ExternalInput").ap()
    bqd = nc.dram_tensor("bq", [2, 128], f32, kind="ExternalInput").ap()
    bkd = nc.dram_tensor("bk", [2, 128], f32, kind="ExternalInput").ap()
    triu2 = nc.dram_tensor("triu2", [128, 256], f32, kind="ExternalInput").ap()
    ident = nc.dram_tensor("ident", [128, 128], bf, kind="ExternalInput").ap()
    po = nc.dram_tensor("po", [D, S], bf, kind="ExternalOutput").ap()

    def mm(out, lhsT, rhs, **kw):
        nc.tensor.matmul(out, lhsT, rhs, **kw)

    with tile.TileContext(nc) as tc:
        with (
            tc.tile_pool(name="persist", bufs=1) as pp,
            tc.tile_pool(name="xin", bufs=3) as xpool,
            tc.tile_pool(name="work", bufs=4) as wk_pool,
            tc.tile_pool(name="outp", bufs=6) as opool,
            tc.tile_pool(name="psA", bufs=3, space="PSUM") as psA,
            tc.tile_pool(name="psB", bufs=5, space="PSUM") as psB,
        ):
            # ---- persistent tiles -------------------------------------------
            wq_sb = pp.tile([128, 2, IB, 128], bf, tag="wq", name="wq_sb")
            wk_sb = pp.tile([128, 2, IB, 128], bf, tag="wk", name="wk_sb")
            wv_sb = pp.tile([128, IB, JS], bf, tag="wv", name="wv_sb")
            wo_sb = pp.tile([128, 2, D], bf, tag="wo", name="wo_sb")
            bq_sb = pp.tile([128, 2], f32, tag="bq", name="bq_sb")
            bk_sb = pp.tile([128, 2], f32, tag="bk", name="bk_sb")
            triu_sb = pp.tile([128, 256], f32, tag="triu", name="triu_sb")
            ident_sb = pp.tile([128, 128], bf, tag="ident", name="ident_sb")

            # q in per-head zero-padded tiles: head hh lives on partitions
            # hh*64..hh*64+64, the other 64 partitions stay zero, so score
            # and Q@S matmuls use a full-128 stationary (shared kT weight
            # load + FWL) without changing the math.  Both heads of a jb
            # share one [128, 2, S] tile so the chunk's scores are a single
            # 256-column matmul.
            qTz = [pp.tile([128, 2, S], bf, tag=f"qTz{jb}", name=f"qTz{jb}")
                   for jb in range(2)]
            kT_sb = [pp.tile([128, S], bf, tag=f"kT{jb}", name=f"kT{jb}") for jb in range(2)]
            aT_c = pp.tile([128, 2, S], bf, tag="aTc", name="aTc")
            vaug_sb = [pp.tile([128, HPC * JAUG], bf, tag=f"vaug{i}", name=f"vaug{i}")
                       for i in range(NCH)]
            # two heads per tile: head h at partitions (h%2)*64 .. +64
            saug_sb = [pp.tile([128, JAUG], f32, tag=f"saug{jb}", name=f"saug{jb}")
                       for jb in range(2)]
            saug_bf = [pp.tile([128, JAUG], bf, tag=f"saugb{jb}", name=f"saugb{jb}")
                       for jb in range(2)]
            attn2_sb = [pp.tile([128, 2 * DEPTH], bf, tag=f"attn2{jb}", name=f"attn2{jb}")
                        for jb in range(2)]

            # zero the unused head half of each qTz tile once (vector is idle
            # during the startup DMA window)
            for jb in range(2):
                for hh in range(2):
                    z0 = (1 - hh) * 64
                    nc.vector.memset(qTz[jb][z0:z0 + 64, hh, :], 0.0)

            # ---- initial loads: q path first so compute starts ASAP ---------
            x_tiles = {}

            def load_x(sc):
                xq_t = xpool.tile([128, 2, IB, HA], bf, tag="xq")
                xk_t = xpool.tile([128, 2, IB, HA], bf, tag="xk")
                xv_t = xpool.tile([128, 4, IB, 128], bf, tag="xv")
                nc.sync.dma_start(xq_t[:], xq[sc])
                nc.scalar.dma_start(xk_t[:], xk[sc])
                nc.sync.dma_start(xv_t[:], xv[sc])
                x_tiles[sc] = (xq_t, xk_t, xv_t)

            # chunk-0 inputs arrive in halves/quarters so the projection
            # matmuls can start as soon as the first slice lands.  sync queue
            # carries the bulky x tensors in consumption order; the scalar
            # queue carries weights so both drain in parallel.
            xq_t0 = xpool.tile([128, 2, IB, HA], bf, tag="xq")
            xk_t0 = xpool.tile([128, 2, IB, HA], bf, tag="xk")
            xv_t0 = xpool.tile([128, 4, IB, 128], bf, tag="xv")
            xq_t1 = xpool.tile([128, 2, IB, HA], bf, tag="xq")
            xk_t1 = xpool.tile([128, 2, IB, HA], bf, tag="xk")
            xv_t1 = xpool.tile([128, 4, IB, 128], bf, tag="xv")
            nc.sync.dma_start(xq_t0[:, 0], xq[0][:, 0])
            nc.scalar.dma_start(wq_sb[:, 0], wq[0])
            nc.scalar.dma_start(bq_sb[:], bqd.rearrange("jb p -> p jb"))
            nc.scalar.dma_start(bk_sb[:], bkd.rearrange("jb p -> p jb"))
            nc.scalar.dma_start(wq_sb[:, 1], wq[1])
            nc.sync.dma_start(xq_t0[:, 1], xq[0][:, 1])
            nc.scalar.dma_start(wk_sb[:, 0], wk[0])
            nc.scalar.dma_start(triu_sb[:], triu2)
            nc.scalar.dma_start(ident_sb[:], ident)
            nc.scalar.dma_start(wk_sb[:, 1], wk[1])
            nc.sync.dma_start(xk_t0[:, 0], xk[0][:, 0])
            nc.sync.dma_start(xk_t0[:, 1], xk[0][:, 1])
            nc.scalar.dma_start(wv_sb[:], wv)
            for ss in range(SCH // 128):
                nc.sync.dma_start(xv_t0[:, ss], xv[0][:, ss])
            # group-1 x interleaved across both queues by first-use time;
            # wo is not needed until the first p3 unit (group 1)
            nc.scalar.dma_start(xk_t1[:], xk[1])
            nc.sync.dma_start(xq_t1[:], xq[1])
            nc.scalar.dma_start(wo_sb[:], wo)
            nc.sync.dma_start(xv_t1[:], xv[1])
            x_tiles[0] = (xq_t0, xk_t0, xv_t0)
            x_tiles[1] = (xq_t1, xk_t1, xv_t1)

            # ---- pumpable work units (one PSUM group each) ------------------
            def write_q(jb, s0, ncols, ps):
                for hh in range(2):
                    r0 = hh * 64
                    nc.scalar.activation(
                        qTz[jb][r0:r0 + 64, hh, s0:s0 + ncols],
                        ps[r0:r0 + 64, 0:ncols],
                        AF.Identity if SIM_NO_PRELU else AF.Prelu,
                        bias=bq_sb[r0:r0 + 64, jb:jb + 1], scale=1.0, alpha=0.1)

            def unit_qk(which, sc, jb):
                s0 = sc * SCH
                x_t = x_tiles[sc][0 if which == "q" else 1]
                w_sb = wq_sb if which == "q" else wk_sb
                ps = psA.tile([128, SCH], f32, tag="A")
                for ib in range(IB):
                    mm(ps[:], w_sb[:, jb, ib, :],
                       x_t[:, :, ib, :],
                       start=(ib == 0), stop=(ib == IB - 1))
                if which == "q":
                    write_q(jb, s0, SCH, ps)
                else:
                    nc.scalar.activation(
                        kT_sb[jb][:, s0:s0 + SCH], ps[:],
                        AF.Identity if SIM_NO_PRELU else AF.Prelu,
                        bias=bk_sb[:, jb:jb + 1], scale=1.0, alpha=0.1)

            def unit_v(sc, ss):
                x_t = x_tiles[sc][2]
                ps = psA.tile([128, JS], f32, tag="A")
                for ib in range(IB):
                    mm(ps[:], x_t[:, ss, ib, :],
                       wv_sb[:, ib, :],
                       start=(ib == 0), stop=(ib == IB - 1))
                vt = vaug_sb[sc * (SCH // 128) + ss]
                vt_r = vt[:].rearrange("p (h e) -> p h e", h=HPC)
                nc.scalar.activation(
                    vt_r[:, :, 0:DEPTH],
                    ps[:].rearrange("p (h e) -> p h e", h=HPC), AF.Copy)
                nc.gpsimd.memset(vt_r[:, :, DEPTH:JAUG], 1.0)

            def unit_qk_half(which, sc, jb, half):
                s0 = sc * SCH + half * (SCH // 2)
                x_t = x_tiles[sc][0 if which == "q" else 1]
                w_sb = wq_sb if which == "q" else wk_sb
                ps = psA.tile([128, SCH // 2], f32, tag="A")
                for ib in range(IB):
                    mm(ps[:], w_sb[:, jb, ib, :],
                       x_t[:, half, ib, :],
                       start=(ib == 0), stop=(ib == IB - 1))
                if which == "q":
                    write_q(jb, s0, SCH // 2, ps)
                else:
                    nc.scalar.activation(
                        kT_sb[jb][:, s0:s0 + SCH // 2], ps[:],
                        AF.Identity if SIM_NO_PRELU else AF.Prelu,
                        bias=bk_sb[:, jb:jb + 1], scale=1.0, alpha=0.1)

            def proj_units(sc, include_v=True):
                u = []
                for jb in range(2):
                    u.append(lambda jb=jb: unit_qk("q", sc, jb))
                    u.append(lambda jb=jb: unit_qk("k", sc, jb))
                if include_v:
                    for ss in range(SCH // 128):
                        u.append(lambda ss=ss: unit_v(sc, ss))
                return u

            def v_units(sc):
                return [lambda ss=ss: unit_v(sc, ss)
                        for ss in range(SCH // 128)]

            def qk_half_units(sc):
                # q/k at half-chunk granularity: 8 units instead of 4, for
                # groups whose pending list would otherwise leave pump slots
                # empty (exposing the chunk dependency chains as PE gaps)
                u = []
                for half in range(2):
                    for jb in range(2):
                        u.append(lambda jb=jb, h=half:
                                 unit_qk_half("q", sc, jb, h))
                        u.append(lambda jb=jb, h=half:
                                 unit_qk_half("k", sc, jb, h))
                return u

            def proj_units0_qk():
                # chunk-0 variant: q/k at half-chunk granularity, ordered to
                # match the staggered arrival of the split input DMAs
                u = []
                for half in range(2):
                    for which in ("q", "k"):
                        for jb in range(2):
                            u.append(lambda w=which, jb=jb, h=half:
                                     unit_qk_half(w, 0, jb, h))
                return u

            po_r = po.rearrange("(ob p) s -> ob p s", p=128)

            def unit_p3(o0, osc, ob, alt=False):
                # alt: after the last attention chunk the psB banks are free,
                # so tail units alternate pools and the PE never waits on a
                # psum WAR behind the trailing copies
                if alt and ob % 2 == 1:
                    ps = psB.tile([128, OSC], f32, tag="B")
                else:
                    ps = psA.tile([128, OSC], f32, tag="A")
                for jb in range(2):
                    mm(ps[:, 0:osc], wo_sb[:, jb, ob * 128:(ob + 1) * 128],
                       aT_c[:, jb, o0:o0 + osc],
                       start=(jb == 0), stop=(jb == 1))
                ot = opool.tile([128, OSC], bf, tag="ot")
                if ob % 2 == 0:
                    nc.vector.tensor_copy(ot[:, 0:osc], ps[:, 0:osc])
                else:
                    nc.scalar.activation(ot[:, 0:osc], ps[:, 0:osc], AF.Copy)
                q_eng = nc.sync if ob % 2 == 0 else nc.scalar
                q_eng.dma_start(po_r[ob, :, o0:o0 + osc], ot[:, 0:osc])

            def p3_units(o0, osc=OSC, alt=False):
                return [lambda ob=ob: unit_p3(o0, osc, ob, alt)
                        for ob in range(D // 128)]

            pending = []
            deferred_tt = []

            def pump():
                if pending:
                    u = pending.pop(0)
                    if u is not None:
                        u()

            def flush_tt():
                while deferred_tt:
                    deferred_tt.pop(0)()

            # ---- attention chunk (pumps a work unit between stages) ---------
            def chunk(ci):
                scol = ci * C
                if ci > 0:
                    nc.vector.tensor_copy(saug_bf[0][:], saug_sb[0][:])
                    nc.gpsimd.tensor_copy(saug_bf[1][:], saug_sb[1][:])
                # previous chunk's attn transposes go first: their input
                # (attn2_sb) settled a full chunk ago, and emitting them
                # before this chunk's DVE work keeps their semaphore waits
                # trivially satisfied (no PE stall behind the DVE backlog)
                flush_tt()

                # stage 1: K transposes (both heads in one op) + scores.
                # kT chunk is the stationary for ONE 256-col score matmul
                # per jb (rhs = both zero-padded per-head q chunks); both
                # jb's scores share a single PSUM bank (the second matmul's
                # start=False region still has has_written clear, so it
                # overwrites rather than accumulates).
                kp2 = psB.tile([128, 2, 2 * DEPTH], bf, tag="B")
                for jb in range(2):
                    nc.tensor.transpose(kp2[:, jb, :],
                                        kT_sb[jb][:, scol:scol + C],
                                        ident_sb[:])
                knat2 = wk_pool.tile([128, 2, 2 * DEPTH], bf, tag="knat")
                nc.vector.tensor_copy(knat2[:], kp2[:])
                at2 = psA.tile([128, 2, 2 * C], f32, tag="A")
                atm = []
                for jb in range(2):
                    mm(at2[:, jb, :], kT_sb[jb][:, scol:scol + C],
                       qTz[jb][:, :, scol:scol + C],
                       start=(jb == 0), stop=(jb == 1),
                       skip_group_check=True)
                for jb in range(2):
                    am = wk_pool.tile([128, 2 * C], bf, tag="atm")
                    nc.vector.tensor_tensor(am[:], at2[:, jb, :],
                                            triu_sb[:], op=Alu.mult)
                    atm.append(am)
                pump()

                # stage 2: numerators + attn, per jb
                for jb in range(2):
                    for hh in range(2):
                        h = jb * 2 + hh
                        vt = vaug_sb[ci][:, h * JAUG:(h + 1) * JAUG]
                        nump = psB.tile([128, JAUG], f32, tag="B")
                        mm(nump[:], atm[jb][:, hh * C:(hh + 1) * C], vt,
                           start=True, stop=(ci == 0))
                        if ci > 0:
                            mm(nump[:], qTz[jb][:, hh, scol:scol + C],
                               saug_bf[jb][:],
                               start=False, stop=True)
                        recip = wk_pool.tile([128, 1], f32, tag="recip")
                        nc.vector.reciprocal(recip[:], nump[:, DEPTH:JAUG])
                        dstap = attn2_sb[jb][:, hh * DEPTH:(hh + 1) * DEPTH]
                        if jb == 0:
                            nc.vector.tensor_scalar_mul(
                                dstap, nump[:, 0:DEPTH], recip[:])
                        else:
                            nc.scalar.activation(dstap, nump[:, 0:DEPTH],
                                                 AF.Copy, scale=recip[:])
                    pump()

                # stage 3: state update S_aug += K^T V_aug (col-tiled pair
                # per jb runs concurrently on disjoint PE column groups;
                # both jb share one PSUM bank via has_written semantics)
                if ci < NCH - 1:
                    d_ps2 = psB.tile([128, 2, JAUG], f32, tag="B")
                    for jb in range(2):
                        for hh in range(2):
                            jo = hh * DEPTH
                            h = jb * 2 + hh
                            vt = vaug_sb[ci][:, h * JAUG:(h + 1) * JAUG]
                            mm(d_ps2[jo:jo + DEPTH, jb, :],
                               knat2[:, jb, jo:jo + DEPTH],
                               vt, start=(jb == 0), stop=(jb == 1),
                               skip_group_check=True)
                    for jb in range(2):
                        if ci == 0:
                            nc.vector.tensor_copy(saug_sb[jb][:],
                                                  d_ps2[:, jb, :])
                        else:
                            nc.vector.tensor_add(saug_sb[jb][:],
                                                 saug_sb[jb][:],
                                                 d_ps2[:, jb, :])

                # stage 4: transpose attn -> aT columns (deferred into the
                # next chunk's stream so the attn-copy chain has cover)
                def tt(scol=scol):
                    at2_ps = psB.tile([128, 2, C], bf, tag="B")
                    for jb in range(2):
                        nc.tensor.transpose(at2_ps[:, jb, :], attn2_sb[jb][:],
                                            ident_sb[:])
                    nc.scalar.activation(aT_c[:, :, scol:scol + C],
                                         at2_ps[:], AF.Copy)
                deferred_tt.append(tt)
                pump()

            # ---- schedule ---------------------------------------------------
            # Each group pumps the NEXT group's projections + the PREVIOUS
            # group's output projection between its attention stages.  The
            # last group's V projection and the last p3 batch are held back
            # so the tail (which otherwise has only small matmuls) keeps the
            # PE clock gate open; the final 512 columns run as one dense
            # block after the final chunk.
            for u in proj_units0_qk():
                u()
            for g in range(NSC):
                if g + 2 < NSC:
                    load_x(g + 2)
                pending = []
                if g == 0:
                    # 16 units for 16 pump slots: group-1 q/k as half-units,
                    # group-1 v at the end (its x arrives last)
                    pending = v_units(0) + qk_half_units(1) + v_units(1)
                elif g == NSC - 1:
                    vs = v_units(g)
                    ps3 = p3_units((g - 1) * OSC)
                    pending = [vs[0], vs[1], ps3[0], vs[2], ps3[1], vs[3]]
                    pending += ps3[2:]
                    # first half of the last group's output projection: its
                    # aT columns (chunks 12-13) are flushed by chunk 14's
                    # stage 1, and these units sit deep enough in `pending`
                    # that they pump no earlier than chunk 15
                    pending += p3_units(g * OSC, osc=OSC // 2)
                else:
                    if g + 1 < NSC - 1:
                        a = proj_units(g + 1)
                    else:
                        # group 3's v is deferred to the tail; its q/k run
                        # as half-units so all 16 pump slots stay filled
                        a = qk_half_units(g + 1)
                    b = p3_units((g - 1) * OSC)
                    while a or b:
                        if a:
                            pending.append(a.pop(0))
                        if b:
                            pending.append(b.pop(0))
                for t in range(CPO):
                    chunk(CPO * g + t)
                while pending:
                    pump()
            flush_tt()
            # tail: the last 256 output columns as one dense block (the
            # first 256 were pumped through chunk 15; copies and stores
            # trail on vector/scalar + both queues)
            for u in p3_units((NSC - 1) * OSC + OSC // 2, osc=OSC // 2,
                              alt=True):
                u()

    nc.compile()
    return nc


def _get_prog():
    global _PROG
    if _PROG is None:
        _PROG = _build()
    return _PROG


def kernel(q, k, v, query_mask, key_mask, value_mask,
           Wq, bq, Wk, bk, Wv, bv, Wo, bo):
    global LAST_RESULTS
    from concourse import bass_utils

    q = np.asarray(q, np.float32)
    k = np.asarray(k, np.float32)
    v = np.asarray(v, np.float32)
    qm = q * np.asarray(query_mask, np.float32)
    km = k * np.asarray(key_mask, np.float32)
    vm = v * np.asarray(value_mask, np.float32)
    Wq = np.asarray(Wq, np.float32)
    Wk = np.asarray(Wk, np.float32)
    Wv = np.asarray(Wv, np.float32)
    Wo = np.asarray(Wo, np.float32)
    bq = np.asarray(bq, np.float32)
    bk = np.asarray(bk, np.float32)
    bv = np.asarray(bv, np.float32)
    bo = np.asarray(bo, np.float32)
    assert not np.any(bv), "kernel assumes bv == 0 (true for this problem)"

    nc = _get_prog()

    triu1 = np.triu(np.ones((128, 128), np.float32))
    triu2 = np.concatenate([triu1, triu1], axis=1)
    ident = np.eye(128, dtype=np.float32).astype(BF16)

    HA = SCH // 2

    def tile_x2(a):  # [S, D] -> [NSC, 128, 2, IB, HA] (half-major)
        return np.ascontiguousarray(
            a.reshape(NSC, 2, HA, IB, 128).transpose(0, 4, 1, 3, 2)).astype(BF16)

    def tile_x4(a):  # [S, D] -> [NSC, 128, 4, IB, 128] (quarter-major)
        return np.ascontiguousarray(
            a.reshape(NSC, 4, 128, IB, 128).transpose(0, 4, 1, 3, 2)).astype(BF16)

    def tile_w(w):  # w: [D, JS] -> [128, IB, JS]
        return w.reshape(IB, 128, JS).transpose(1, 0, 2).astype(BF16)

    def tile_w2(w):  # w: [D, JS] -> [2, 128, IB, 128] (jb-major)
        return np.ascontiguousarray(
            w.reshape(IB, 128, 2, 128).transpose(2, 1, 0, 3)).astype(BF16)

    xqs = [tile_x2(qm[b]) for b in range(B)]
    xks = [tile_x2(km[b]) for b in range(B)]
    xvs = [tile_x4(vm[b]) for b in range(B)]

    in_maps = []
    for c in range(N_CORES):
        b, g = divmod(c, HPC)
        js = slice(g * JS, (g + 1) * JS)
        in_maps.append({
            "xq": xqs[b], "xk": xks[b], "xv": xvs[b],
            "wq": tile_w2(Wq[:, js]),
            "wk": tile_w2(Wk[:, js]),
            "wv": tile_w(Wv[:, js]),
            "wo": Wo[js, :].reshape(2, 128, D).transpose(1, 0, 2).astype(BF16),
            "bq": np.ascontiguousarray(bq[js].reshape(2, 128)),
            "bk": np.ascontiguousarray(bk[js].reshape(2, 128)),
            "triu2": triu2, "ident": ident,
        })

    res = bass_utils.run_bass_kernel_spmd(
        nc, in_maps, core_ids=list(range(N_CORES)),
        trace=TRACE, trace_cores=TRACE_CORES)
    LAST_RESULTS = res

    out = np.zeros((B, S, D), np.float32)
    for c in range(N_CORES):
        out[c // HPC] += res.results[c]["po"].astype(np.float32).T
    out += bo
    return out


# revision 53
# speedup vs baseline: 1.1999x; 1.0084x over previous
"""Trainium2 Bass kernel for nn_MultiHeadAttention_KT (causal linear attention).

Math (per batch b):
  q' = leaky((q*qm) @ Wq + bq); k' = leaky((k*km) @ Wk + bk); v' = (v*vm) @ Wv
  per head h (DEPTH=64):   S_t = sum_{s<=t} k_s v_s^T ; z_t = sum_{s<=t} k_s
                           attn_t = (q_t @ S_t) / (q_t . z_t)
  out = concat_heads(attn) @ Wo + bo
Sharding: 8 cores = 2 batches x 4 head-groups (4 heads / 256 cols each).
Host transposes + bf16-casts inputs (xq = (q*qm)^T etc.); host sums the 4
partial output projections per batch (po is this core's heads' Wo slice).

All matmul operands are bf16 (1 cyc/row on PE + fast weight load); PSUM
accumulation stays f32.  Chunked linear attention (chunk C=128):
  AT   = K Q^T (per chunk, [s,t] layout)      masked with triu (s<=t)
  num  = ATm^T V_aug + Q S_aug                (V_aug = [V | 1], S_aug = [S | z])
  attn = num[:, :64] * (1/num[:, 64])
  S_aug += K_chunk^T V_aug                    (delta matmul; f32 master state
                                               on DVE + bf16 mirror)

q' is stored in per-head zero-padded tiles (qTz[jb][hh], [128, S] with the
other head's 64 partitions zeroed) so every attention matmul has a 128-tall
stationary: scores share one kT LDWEIGHTS for both heads and all stationaries
qualify for fast weight load, which keeps the small-matmul issue rate near
the stream rate.  Projection / output-projection units are pumped one at a
time between attention stages so the PE HAM clock gate stays open; the last
group's V projection and the previous group's output projection are held
back to feed the tail, where no other big matmuls remain.
"""

import os
import sys

sys.path.insert(0, "/opt/trn_rl_repo")

import ml_dtypes
import numpy as np

BF16 = np.dtype(ml_dtypes.bfloat16)

B, S, D, H = 2, 2048, 1024, 16
DEPTH = 64
N_CORES = 8
HPC = 4                 # heads per core
JS = HPC * DEPTH        # 256 projected columns per core
C = 128                 # attention chunk length
NCH = S // C            # 16 chunks
IB = D // 128           # 8 contraction blocks
SCH = 512               # projection s-chunk
NSC = S // SCH          # 4 projection chunks
JAUG = DEPTH + 1        # 65 (V augmented with ones column)
OSC = 512               # output projection s-chunk
CPO = OSC // C          # attention chunks per output chunk

MM_DTYPE = "bf16"       # informational (printed by test harness)
SIM_NO_PRELU = os.environ.get("KT_SIM_NO_PRELU") == "1"  # CoreSim lacks Prelu
TRACE = False           # set True from test harness to capture NTFF profile
TRACE_CORES = None
LAST_RESULTS = None     # BassKernelResults of the last kernel() call

_PROG = None


def _build():
    import concourse.bacc as bacc
    import concourse.mybir as mybir
    import concourse.tile as tile

    dt = mybir.dt
    f32 = dt.float32
    bf = dt.bfloat16
    AF = mybir.ActivationFunctionType
    Alu = mybir.AluOpType

    nc = bacc.Bacc("TRN2", target_bir_lowering=False, debug=False,
                   num_devices=N_CORES)

    # host pre-tiles x and weights so every DMA is per-partition contiguous.
    # xq/xk are half-major and xv quarter-major so the startup split loads
    # are single 4KB/2KB descriptor rows instead of 512B fragments (the DMA
    # engines are descriptor-rate bound at kernel start).
    HA = SCH // 2
    xq = nc.dram_tensor("xq", [NSC, 128, 2, IB, HA], bf, kind="ExternalInput").ap()
    xk = nc.dram_tensor("xk", [NSC, 128, 2, IB, HA], bf, kind="ExternalInput").ap()
    xv = nc.dram_tensor("xv", [NSC, 128, 4, IB, 128], bf, kind="ExternalInput").ap()
    wq = nc.dram_tensor("wq", [2, 128, IB, 128], bf, kind="ExternalInput").ap()
    wk = nc.dram_tensor("wk", [2, 128, IB, 128], bf, kind="ExternalInput").ap()
    wv = nc.dram_tensor("wv", [128, IB, JS], bf, kind="ExternalInput").ap()
    wo = nc.dram_tensor("wo", [128, 2, D], bf, kind="ExternalInput").ap()
    bqd = nc.dram_tensor("bq", [2, 128], f32, kind="ExternalInput").ap()
    bkd = nc.dram_tensor("bk", [2, 128], f32, kind="ExternalInput").ap()
    triu2 = nc.dram_tensor("triu2", [128, 256], f32, kind="ExternalInput").ap()
    ident = nc.dram_tensor("ident", [128, 128], bf, kind="ExternalInput").ap()
    po = nc.dram_tensor("po", [D, S], bf, kind="ExternalOutput").ap()

    def mm(out, lhsT, rhs, **kw):
        nc.tensor.matmul(out, lhsT, rhs, **kw)

    with tile.TileContext(nc) as tc:
        with (
            tc.tile_pool(name="persist", bufs=1) as pp,
            tc.tile_pool(name="xin", bufs=3) as xpool,
            tc.tile_pool(name="work", bufs=4) as wk_pool,
            tc.tile_pool(name="outp", bufs=6) as opool,
            tc.tile_pool(name="psA", bufs=3, space="PSUM") as psA,
            tc.tile_pool(name="psB", bufs=5, space="PSUM") as psB,
        ):
            # ---- persistent tiles -------------------------------------------
            wq_sb = pp.tile([128, 2, IB, 128], bf, tag="wq", name="wq_sb")
            wk_sb = pp.tile([128, 2, IB, 128], bf, tag="wk", name="wk_sb")
            wv_sb = pp.tile([128, IB, JS], bf, tag="wv", name="wv_sb")
            wo_sb = pp.tile([128, 2, D], bf, tag="wo", name="wo_sb")
            bq_sb = pp.tile([128, 2], f32, tag="bq", name="bq_sb")
            bk_sb = pp.tile([128, 2], f32, tag="bk", name="bk_sb")
            triu_sb = pp.tile([128, 256], f32, tag="triu", name="triu_sb")
            ident_sb = pp.tile([128, 128], bf, tag="ident", name="ident_sb")

            # q in per-head zero-padded tiles: head hh lives on partitions
            # hh*64..hh*64+64, the other 64 partitions stay zero, so score
            # and Q@S matmuls use a full-128 stationary (shared kT weight
            # load + FWL) without changing the math.  Both heads of a jb
            # share one [128, 2, S] tile so the chunk's scores are a single
            # 256-column matmul.
            qTz = [pp.tile([128, 2, S], bf, tag=f"qTz{jb}", name=f"qTz{jb}")
                   for jb in range(2)]
            kT_sb = [pp.tile([128, S], bf, tag=f"kT{jb}", name=f"kT{jb}") for jb in range(2)]
            aT_c = pp.tile([128, 2, S], bf, tag="aTc", name="aTc")
            vaug_sb = [pp.tile([128, HPC * JAUG], bf, tag=f"vaug{i}", name=f"vaug{i}")
                       for i in range(NCH)]
            # two heads per tile: head h at partitions (h%2)*64 .. +64
            saug_sb = [pp.tile([128, JAUG], f32, tag=f"saug{jb}", name=f"saug{jb}")
                       for jb in range(2)]
            saug_bf = [pp.tile([128, JAUG], bf, tag=f"saugb{jb}", name=f"saugb{jb}")
                       for jb in range(2)]
            attn2_sb = [pp.tile([128, 2 * DEPTH], bf, tag=f"attn2{jb}", name=f"attn2{jb}")
                        for jb in range(2)]

            # zero the unused head half of each qTz tile once (vector is idle
            # during the startup DMA window)
            for jb in range(2):
                for hh in range(2):
                    z0 = (1 - hh) * 64
                    nc.vector.memset(qTz[jb][z0:z0 + 64, hh, :], 0.0)

            # ---- initial loads: q path first so compute starts ASAP ---------
            x_tiles = {}

            def load_x(sc):
                xq_t = xpool.tile([128, 2, IB, HA], bf, tag="xq")
                xk_t = xpool.tile([128, 2, IB, HA], bf, tag="xk")
                xv_t = xpool.tile([128, 4, IB, 128], bf, tag="xv")
                nc.sync.dma_start(xq_t[:], xq[sc])
                nc.scalar.dma_start(xk_t[:], xk[sc])
                nc.sync.dma_start(xv_t[:], xv[sc])
                x_tiles[sc] = (xq_t, xk_t, xv_t)

            # chunk-0 inputs arrive in halves/quarters so the projection
            # matmuls can start as soon as the first slice lands.  sync queue
            # carries the bulky x tensors in consumption order; the scalar
            # queue carries weights so both drain in parallel.
            xq_t0 = xpool.tile([128, 2, IB, HA], bf, tag="xq")
            xk_t0 = xpool.tile([128, 2, IB, HA], bf, tag="xk")
            xv_t0 = xpool.tile([128, 4, IB, 128], bf, tag="xv")
            xq_t1 = xpool.tile([128, 2, IB, HA], bf, tag="xq")
            xk_t1 = xpool.tile([128, 2, IB, HA], bf, tag="xk")
            xv_t1 = xpool.tile([128, 4, IB, 128], bf, tag="xv")
            nc.sync.dma_start(xq_t0[:, 0], xq[0][:, 0])
            nc.scalar.dma_start(wq_sb[:, 0], wq[0])
            nc.scalar.dma_start(bq_sb[:], bqd.rearrange("jb p -> p jb"))
            nc.scalar.dma_start(bk_sb[:], bkd.rearrange("jb p -> p jb"))
            nc.scalar.dma_start(wq_sb[:, 1], wq[1])
            nc.sync.dma_start(xq_t0[:, 1], xq[0][:, 1])
            nc.scalar.dma_start(wk_sb[:, 0], wk[0])
            nc.scalar.dma_start(triu_sb[:], triu2)
            nc.scalar.dma_start(ident_sb[:], ident)
            nc.scalar.dma_start(wk_sb[:, 1], wk[1])
            nc.sync.dma_start(xk_t0[:, 0], xk[0][:, 0])
            nc.sync.dma_start(xk_t0[:, 1], xk[0][:, 1])
            nc.scalar.dma_start(wv_sb[:], wv)
            for ss in range(SCH // 128):
                nc.sync.dma_start(xv_t0[:, ss], xv[0][:, ss])
            # group-1 x interleaved across both queues by first-use time;
            # wo is not needed until the first p3 unit (group 1)
            nc.scalar.dma_start(xk_t1[:], xk[1])
            nc.sync.dma_start(xq_t1[:], xq[1])
            nc.scalar.dma_start(wo_sb[:], wo)
            nc.sync.dma_start(xv_t1[:], xv[1])
            x_tiles[0] = (xq_t0, xk_t0, xv_t0)
            x_tiles[1] = (xq_t1, xk_t1, xv_t1)

            # ---- pumpable work units (one PSUM group each) ------------------
            def write_q(jb, s0, ncols, ps):
                for hh in range(2):
                    r0 = hh * 64
                    nc.scalar.activation(
                        qTz[jb][r0:r0 + 64, hh, s0:s0 + ncols],
                        ps[r0:r0 + 64, 0:ncols],
                        AF.Identity if SIM_NO_PRELU else AF.Prelu,
                        bias=bq_sb[r0:r0 + 64, jb:jb + 1], scale=1.0, alpha=0.1)

            def unit_qk(which, sc, jb):
                s0 = sc * SCH
                x_t = x_tiles[sc][0 if which == "q" else 1]
                w_sb = wq_sb if which == "q" else wk_sb
                ps = psA.tile([128, SCH], f32, tag="A")
                for ib in range(IB):
                    mm(ps[:], w_sb[:, jb, ib, :],
                       x_t[:, :, ib, :],
                       start=(ib == 0), stop=(ib == IB - 1))
                if which == "q":
                    write_q(jb, s0, SCH, ps)
                else:
                    nc.scalar.activation(
                        kT_sb[jb][:, s0:s0 + SCH], ps[:],
                        AF.Identity if SIM_NO_PRELU else AF.Prelu,
                        bias=bk_sb[:, jb:jb + 1], scale=1.0, alpha=0.1)

            def unit_v(sc, ss):
                x_t = x_tiles[sc][2]
                ps = psA.tile([128, JS], f32, tag="A")
                for ib in range(IB):
                    mm(ps[:], x_t[:, ss, ib, :],
                       wv_sb[:, ib, :],
                       start=(ib == 0), stop=(ib == IB - 1))
                vt = vaug_sb[sc * (SCH // 128) + ss]
                vt_r = vt[:].rearrange("p (h e) -> p h e", h=HPC)
                nc.scalar.activation(
                    vt_r[:, :, 0:DEPTH],
                    ps[:].rearrange("p (h e) -> p h e", h=HPC), AF.Copy)
                nc.gpsimd.memset(vt_r[:, :, DEPTH:JAUG], 1.0)

            def unit_qk_half(which, sc, jb, half):
                s0 = sc * SCH + half * (SCH // 2)
                x_t = x_tiles[sc][0 if which == "q" else 1]
                w_sb = wq_sb if which == "q" else wk_sb
                ps = psA.tile([128, SCH // 2], f32, tag="A")
                for ib in range(IB):
                    mm(ps[:], w_sb[:, jb, ib, :],
                       x_t[:, half, ib, :],
                       start=(ib == 0), stop=(ib == IB - 1))
                if which == "q":
                    write_q(jb, s0, SCH // 2, ps)
                else:
                    nc.scalar.activation(
                        kT_sb[jb][:, s0:s0 + SCH // 2], ps[:],
                        AF.Identity if SIM_NO_PRELU else AF.Prelu,
                        bias=bk_sb[:, jb:jb + 1], scale=1.0, alpha=0.1)

            def proj_units(sc, include_v=True):
                u = []
                for jb in range(2):
                    u.append(lambda jb=jb: unit_qk("q", sc, jb))
                    u.append(lambda jb=jb: unit_qk("k", sc, jb))
                if include_v:
                    for ss in range(SCH // 128):
                        u.append(lambda ss=ss: unit_v(sc, ss))
                return u

            def v_units(sc):
                return [lambda ss=ss: unit_v(sc, ss)
                        for ss in range(SCH // 128)]

            def qk_half_units(sc):
                # q/k at half-chunk granularity: 8 units instead of 4, for
                # groups whose pending list would otherwise leave pump slots
                # empty (exposing the chunk dependency chains as PE gaps)
                u = []
                for half in range(2):
                    for jb in range(2):
                        u.append(lambda jb=jb, h=half:
                                 unit_qk_half("q", sc, jb, h))
                        u.append(lambda jb=jb, h=half:
                                 unit_qk_half("k", sc, jb, h))
                return u

            def proj_units0_qk():
                # chunk-0 variant: q/k at half-chunk granularity, ordered to
                # match the staggered arrival of the split input DMAs
                u = []
                for half in range(2):
                    for which in ("q", "k"):
                        for jb in range(2):
                            u.append(lambda w=which, jb=jb, h=half:
                                     unit_qk_half(w, 0, jb, h))
                return u

            po_r = po.rearrange("(ob p) s -> ob p s", p=128)

            def unit_p3(o0, osc, ob, alt=False):
                # alt: after the last attention chunk the psB banks are free,
                # so tail units alternate pools and the PE never waits on a
                # psum WAR behind the trailing copies
                if alt and ob % 2 == 1:
                    ps = psB.tile([128, OSC], f32, tag="B")
                else:
                    ps = psA.tile([128, OSC], f32, tag="A")
                for jb in range(2):
                    mm(ps[:, 0:osc], wo_sb[:, jb, ob * 128:(ob + 1) * 128],
                       aT_c[:, jb, o0:o0 + osc],
                       start=(jb == 0), stop=(jb == 1))
                ot = opool.tile([128, OSC], bf, tag="ot")
                if ob % 2 == 0:
                    nc.vector.tensor_copy(ot[:, 0:osc], ps[:, 0:osc])
                else:
                    nc.scalar.activation(ot[:, 0:osc], ps[:, 0:osc], AF.Copy)
                q_eng = nc.sync if ob % 2 == 0 else nc.scalar
                q_eng.dma_start(po_r[ob, :, o0:o0 + osc], ot[:, 0:osc])

            def p3_units(o0, osc=OSC, alt=False):
                return [lambda ob=ob: unit_p3(o0, osc, ob, alt)
                        for ob in range(D // 128)]

            pending = []
            deferred_tt = []

            def pump():
                if pending:
                    u = pending.pop(0)
                    if u is not None:
                        u()

            def flush_tt():
                while deferred_tt:
                    deferred_tt.pop(0)()

            # ---- attention chunk (pumps a work unit between stages) ---------
            def chunk(ci):
                scol = ci * C
                if ci > 0:
                    nc.vector.tensor_copy(saug_bf[0][:], saug_sb[0][:])
                    nc.gpsimd.tensor_copy(saug_bf[1][:], saug_sb[1][:])
                # previous chunk's attn transposes go first: their input
                # (attn2_sb) settled a full chunk ago, and emitting them
                # before this chunk's DVE work keeps their semaphore waits
                # trivially satisfied (no PE stall behind the DVE backlog)
                flush_tt()

                # stage 1: K transposes (both heads in one op) + scores.
                # kT chunk is the stationary for ONE 256-col score matmul
                # per jb (rhs = both zero-padded per-head q chunks); both
                # jb's scores share a single PSUM bank (the second matmul's
                # start=False region still has has_written clear, so it
                # overwrites rather than accumulates).
                kp2 = psB.tile([128, 2, 2 * DEPTH], bf, tag="B")
                for jb in range(2):
                    nc.tensor.transpose(kp2[:, jb, :],
                                        kT_sb[jb][:, scol:scol + C],
                                        ident_sb[:])
                knat2 = wk_pool.tile([128, 2, 2 * DEPTH], bf, tag="knat")
                nc.vector.tensor_copy(knat2[:], kp2[:])
                at2 = psA.tile([128, 2, 2 * C], f32, tag="A")
                atm = []
                for jb in range(2):
                    mm(at2[:, jb, :], kT_sb[jb][:, scol:scol + C],
                       qTz[jb][:, :, scol:scol + C],
                       start=(jb == 0), stop=(jb == 1),
                       skip_group_check=True)
                for jb in range(2):
                    am = wk_pool.tile([128, 2 * C], bf, tag="atm")
                    nc.vector.tensor_tensor(am[:], at2[:, jb, :],
                                            triu_sb[:], op=Alu.mult)
                    atm.append(am)
                pump()

                # stage 2: numerators + attn, per jb
                for jb in range(2):
                    for hh in range(2):
                        h = jb * 2 + hh
                        vt = vaug_sb[ci][:, h * JAUG:(h + 1) * JAUG]
                        nump = psB.tile([128, JAUG], f32, tag="B")
                        mm(nump[:], atm[jb][:, hh * C:(hh + 1) * C], vt,
                           start=True, stop=(ci == 0))
                        if ci > 0:
                            mm(nump[:], qTz[jb][:, hh, scol:scol + C],
                               saug_bf[jb][:],
                               start=False, stop=True)
                        recip = wk_pool.tile([128, 1], f32, tag="recip")
                        nc.vector.reciprocal(recip[:], nump[:, DEPTH:JAUG])
                        dstap = attn2_sb[jb][:, hh * DEPTH:(hh + 1) * DEPTH]
                        if jb == 0:
                            nc.vector.tensor_scalar_mul(
                                dstap, nump[:, 0:DEPTH], recip[:])
                        else:
                            nc.scalar.activation(dstap, nump[:, 0:DEPTH],
                                                 AF.Copy, scale=recip[:])
                    pump()

                # stage 3: state update S_aug += K^T V_aug (col-tiled pair
                # per jb runs concurrently on disjoint PE column groups;
                # both jb share one PSUM bank via has_written semantics)
                if ci < NCH - 1:
                    d_ps2 = psB.tile([128, 2, JAUG], f32, tag="B")
                    for jb in range(2):
                        for hh in range(2):
                            jo = hh * DEPTH
                            h = jb * 2 + hh
                            vt = vaug_sb[ci][:, h * JAUG:(h + 1) * JAUG]
                            mm(d_ps2[jo:jo + DEPTH, jb, :],
                               knat2[:, jb, jo:jo + DEPTH],
                               vt, start=(jb == 0), stop=(jb == 1),
                               skip_group_check=True)
                    for jb in range(2):
                        if ci == 0:
                            nc.vector.tensor_copy(saug_sb[jb][:],
                                                  d_ps2[:, jb, :])
                        else:
                            nc.vector.tensor_add(saug_sb[jb][:],
                                                 saug_sb[jb][:],
                                                 d_ps2[:, jb, :])

                # stage 4: transpose attn -> aT columns (deferred into the
                # next chunk's stream so the attn-copy chain has cover)
                def tt(scol=scol):
                    at2_ps = psB.tile([128, 2, C], bf, tag="B")
                    for jb in range(2):
                        nc.tensor.transpose(at2_ps[:, jb, :], attn2_sb[jb][:],
                                            ident_sb[:])
                    nc.scalar.activation(aT_c[:, :, scol:scol + C],
                                         at2_ps[:], AF.Copy)
                deferred_tt.append(tt)
                pump()

            # ---- schedule ---------------------------------------------------
            # Each group pumps the NEXT group's projections + the PREVIOUS
            # group's output projection between its attention stages.  The
            # last group's V projection and the last p3 batch are held back
            # so the tail (which otherwise has only small matmuls) keeps the
            # PE clock gate open; the final 512 columns run as one dense
            # block after the final chunk.
            for u in proj_units0_qk():
                u()
            for g in range(NSC):
                if g + 2 < NSC:
                    load_x(g + 2)
                pending = []
                if g == 0:
                    # 16 units for 16 pump slots: group-1 q/k as half-units,
                    # group-1 v at the end (its x arrives last)
                    pending = v_units(0) + qk_half_units(1) + v_units(1)
                elif g == NSC - 1:
                    vs = v_units(g)
                    ps3 = p3_units((g - 1) * OSC)
                    pending = [vs[0], vs[1], ps3[0], vs[2], ps3[1], vs[3]]
                    pending += ps3[2:]
                    # first half of the last group's output projection: its
                    # aT columns (chunks 12-13) are flushed by chunk 14's
                    # stage 1, and these units sit deep enough in `pending`
                    # that they pump no earlier than chunk 15
                    pending += p3_units(g * OSC, osc=OSC // 2)
                else:
                    if g + 1 < NSC - 1:
                        a = proj_units(g + 1)
                    else:
                        # group 3's v is deferred to the tail; its q/k run
                        # as half-units so all 16 pump slots stay filled
                        a = qk_half_units(g + 1)
                    b = p3_units((g - 1) * OSC)
                    while a or b:
                        if a:
                            pending.append(a.pop(0))
                        if b:
                            pending.append(b.pop(0))
                for t in range(CPO):
                    chunk(CPO * g + t)
                if g == NSC - 1:
                    # emit the last chunk's attn transposes + aT copy now so
                    # the leftover pending units cover their latency; the
                    # final p3 block then starts with its aT columns ready
                    flush_tt()
                while pending:
                    pump()
            flush_tt()
            # tail: the last 256 output columns as one dense block (the
            # first 256 were pumped through chunk 15; copies and stores
            # trail on vector/scalar + both queues)
            for u in p3_units((NSC - 1) * OSC + OSC // 2, osc=OSC // 2,
                              alt=True):
                u()

    nc.compile()
    return nc


def _get_prog():
    global _PROG
    if _PROG is None:
        _PROG = _build()
    return _PROG


def kernel(q, k, v, query_mask, key_mask, value_mask,
           Wq, bq, Wk, bk, Wv, bv, Wo, bo):
    global LAST_RESULTS
    from concourse import bass_utils

    q = np.asarray(q, np.float32)
    k = np.asarray(k, np.float32)
    v = np.asarray(v, np.float32)
    qm = q * np.asarray(query_mask, np.float32)
    km = k * np.asarray(key_mask, np.float32)
    vm = v * np.asarray(value_mask, np.float32)
    Wq = np.asarray(Wq, np.float32)
    Wk = np.asarray(Wk, np.float32)
    Wv = np.asarray(Wv, np.float32)
    Wo = np.asarray(Wo, np.float32)
    bq = np.asarray(bq, np.float32)
    bk = np.asarray(bk, np.float32)
    bv = np.asarray(bv, np.float32)
    bo = np.asarray(bo, np.float32)
    assert not np.any(bv), "kernel assumes bv == 0 (true for this problem)"

    nc = _get_prog()

    triu1 = np.triu(np.ones((128, 128), np.float32))
    triu2 = np.concatenate([triu1, triu1], axis=1)
    ident = np.eye(128, dtype=np.float32).astype(BF16)

    HA = SCH // 2

    def tile_x2(a):  # [S, D] -> [NSC, 128, 2, IB, HA] (half-major)
        return np.ascontiguousarray(
            a.reshape(NSC, 2, HA, IB, 128).transpose(0, 4, 1, 3, 2)).astype(BF16)

    def tile_x4(a):  # [S, D] -> [NSC, 128, 4, IB, 128] (quarter-major)
        return np.ascontiguousarray(
            a.reshape(NSC, 4, 128, IB, 128).transpose(0, 4, 1, 3, 2)).astype(BF16)

    def tile_w(w):  # w: [D, JS] -> [128, IB, JS]
        return w.reshape(IB, 128, JS).transpose(1, 0, 2).astype(BF16)

    def tile_w2(w):  # w: [D, JS] -> [2, 128, IB, 128] (jb-major)
        return np.ascontiguousarray(
            w.reshape(IB, 128, 2, 128).transpose(2, 1, 0, 3)).astype(BF16)

    xqs = [tile_x2(qm[b]) for b in range(B)]
    xks = [tile_x2(km[b]) for b in range(B)]
    xvs = [tile_x4(vm[b]) for b in range(B)]

    in_maps = []
    for c in range(N_CORES):
        b, g = divmod(c, HPC)
        js = slice(g * JS, (g + 1) * JS)
        in_maps.append({
            "xq": xqs[b], "xk": xks[b], "xv": xvs[b],
            "wq": tile_w2(Wq[:, js]),
            "wk": tile_w2(Wk[:, js]),
            "wv": tile_w(Wv[:, js]),
            "wo": Wo[js, :].reshape(2, 128, D).transpose(1, 0, 2).astype(BF16),
            "bq": np.ascontiguousarray(bq[js].reshape(2, 128)),
            "bk": np.ascontiguousarray(bk[js].reshape(2, 128)),
            "triu2": triu2, "ident": ident,
        })

    res = bass_utils.run_bass_kernel_spmd(
        nc, in_maps, core_ids=list(range(N_CORES)),
        trace=TRACE, trace_cores=TRACE_CORES)
    LAST_RESULTS = res

    out = np.zeros((B, S, D), np.float32)
    for c in range(N_CORES):
        out[c // HPC] += res.results[c]["po"].astype(np.float32).T
    out += bo
    return out
